# revision 68
# baseline (speedup 1.0000x reference)
"""Expert-choice MoE router kernel for Trainium2 (8 NeuronCores), v3.

Problem (B=4, T=8192, D=512, E=8, H=2048, C=1024):
  scores = x @ Wg; w = softmax over T per (b,e); top-C tokens per (b,e);
  y = gelu(x[sel] @ W1) @ W2 * w[sel]; out = scatter_add(y)/max(sum w, 1e-8)

Sharding: expert-parallel, one expert per core.

v3 = v1's proven selection/compaction machinery + the v2 wins that were
individually validated on HW:
  - dense accumulator: bf16, split per batch (4 tensors), zeroed on the
    scalar-engine DMA queue (v1 lost 200 us blocking the sync queue).
  - ReduceScatter: bf16, per batch, issued inside the FFN loop so RS(b)
    overlaps FFN(b+1).
  - bisection: 28 rounds over [-8, 8] (max|score| ~4.9, top-C gap 3.7e-6
    >> 16/2^28), one PE trip per round via block-diag segment-sum.
  - compaction: v1 verbatim (two sparse_gathers per batch on (16, 512)
    slices + selection-matmul relayout) -- the v2 paired variant crashes
    the exec unit on HW.
"""

import sys
from dataclasses import dataclass

sys.path.insert(0, "/opt/trn_rl_repo")

import numpy as np
import ml_dtypes

import concourse.bass as bass  # noqa: F401
import concourse.mybir as mybir
import concourse.tile as tile
from concourse import bacc
from concourse.bass import IndirectOffsetOnAxis
from concourse.bass_utils import run_bass_kernel_spmd

F32 = mybir.dt.float32
BF16 = mybir.dt.bfloat16
I32 = mybir.dt.int32
U32 = mybir.dt.uint32
AF = mybir.ActivationFunctionType
ALU = mybir.AluOpType

NCORES = 8


@dataclass(frozen=True)
class Cfg:
    B: int = 4
    T: int = 8192
    D: int = 512
    E: int = 8
    H: int = 2048
    C: int = 1024
    nrounds: int = 28
    span: float = 8.0
    act: str = "Gelu"
    acc_bf16: bool = True
    rs_inline: bool = True
    stage: int = 4   # 1=+bisect, 2=+selection, 3=+FFN, 4=full

    @property
    def BT(self):
        return self.B * self.T

    @property
    def TSH(self):
        return self.BT // NCORES

    @property
    def ROW(self):
        return self.D + 8

    @property
    def DC(self):
        return self.D // 128

    @property
    def HC(self):
        return self.H // 128

    @property
    def TPP(self):
        return self.T * self.B // 128

    @property
    def RPB(self):
        return NCORES // self.B

    @property
    def QL(self):
        return self.T // self.RPB // 16

    @property
    def TB16(self):
        return self.T // 16

    @property
    def CF(self):
        return self.C // 16

    @property
    def CS(self):
        return self.C // 128


FULL = Cfg()


def build_nc(cfg: Cfg = FULL):
    B, T, D, E, H, C = cfg.B, cfg.T, cfg.D, cfg.E, cfg.H, cfg.C
    BT, TSH, ROW, DC, HC = cfg.BT, cfg.TSH, cfg.ROW, cfg.DC, cfg.HC
    TPP, RPB, QL, TB16 = cfg.TPP, cfg.RPB, cfg.QL, cfg.TB16
    CF, CS = cfg.CF, cfg.CS
    NT = 512
    TQ = 1024               # rows per core of a per-batch ReduceScatter
    ACC = BF16 if cfg.acc_bf16 else F32

    nc = bacc.Bacc("TRN2", target_bir_lowering=False, debug=False,
                   num_devices=NCORES)

    # ---- I/O ----
    x_bf = nc.dram_tensor("x_bf", [BT, D], BF16, kind="ExternalInput")
    xt_sh = nc.dram_tensor("xt_sh", [D, TSH], F32, kind="ExternalInput")
    wg_d = nc.dram_tensor("wg", [D, E], F32, kind="ExternalInput")
    w1_d = nc.dram_tensor("w1", [D, H], BF16, kind="ExternalInput")
    w2_d = nc.dram_tensor("w2", [H, D], BF16, kind="ExternalInput")
    blk_d = nc.dram_tensor("blk128", [128, 128], F32, kind="ExternalInput")
    e1b_d = nc.dram_tensor("e1b", [128, B], F32, kind="ExternalInput")
    iotap1_d = nc.dram_tensor("iotap1", [16, B * TB16], F32,
                              kind="ExternalInput")
    id4_d = nc.dram_tensor("id4", [B, B], I32, kind="ExternalInput")
    o416_d = nc.dram_tensor("o416", [B, 16], F32, kind="ExternalInput")
    id16_d = nc.dram_tensor("id16", [16, 16], F32, kind="ExternalInput")
    idbf_d = nc.dram_tensor("idbf", [128, 128], BF16, kind="ExternalInput")
    o16_d = nc.dram_tensor("o16", [16, 1], F32, kind="ExternalInput")
    mk_d = nc.dram_tensor("mk", [CF, 128], F32, kind="ExternalInput")
    rsel_d = nc.dram_tensor("rsel", [CF, CS], F32, kind="ExternalInput")

    out_sh = nc.dram_tensor("out_sh", [TSH, D], F32, kind="ExternalOutput")
    nf_out = nc.dram_tensor("nf_out", [B, 2], U32, kind="ExternalOutput")

    # ---- internal DRAM ----
    a2a_in = nc.dram_tensor("a2a_in", [E, TSH], F32)
    a2a_out = nc.dram_tensor("a2a_out", [E, TSH], F32)
    dense_b = [nc.dram_tensor(f"dense{b}", [T, ROW], ACC) for b in range(B)]
    rs_b = [nc.dram_tensor(f"rs{b}", [TQ, ROW], ACC) for b in range(B)]

    with tile.TileContext(nc) as tc:
        with (
            tc.tile_pool(name="const", bufs=1) as cp,
            tc.tile_pool(name="sc", bufs=2) as scp,
            tc.tile_pool(name="bis", bufs=1) as bp,
            tc.tile_pool(name="ffn", bufs=2) as fp,
            tc.tile_pool(name="pk", bufs=2 if cfg.acc_bf16 else 1) as pkp,
            tc.tile_pool(name="norm", bufs=2) as np_,
            tc.tile_pool(name="pmm", bufs=2, space="PSUM") as pmm,
            tc.tile_pool(name="pps", bufs=3, space="PSUM") as pps,
        ):
            # ---------- phase 0: zero dense accumulators (scalar queue) ---
            ZR = 8 if cfg.acc_bf16 else 4
            zt = cp.tile([128, ZR * ROW], ACC, tag="zt")
            nc.vector.memset(zt[:], 0.0)
            for b in range(B):
                dz = dense_b[b].ap().rearrange(
                    "(j p zr) r -> j p (zr r)", p=128, zr=ZR)
                for j in range(T // (128 * ZR)):
                    nc.scalar.dma_start(dz[j], zt[:])

            # ---------- load constants / weights ----------
            wg_sb = cp.tile([128, DC, E], F32, tag="wg_sb")
            nc.sync.dma_start(wg_sb[:], wg_d.ap().rearrange("(c p) e -> p c e", p=128))
            w1_sb = cp.tile([128, DC, H], BF16, tag="w1_sb")
            nc.sync.dma_start(w1_sb[:], w1_d.ap().rearrange("(c p) h -> p c h", p=128))
            w2_sb = cp.tile([128, HC, D], BF16, tag="w2_sb")
            nc.sync.dma_start(w2_sb[:], w2_d.ap().rearrange("(c p) d -> p c d", p=128))
            blks = cp.tile([128, 128], F32, tag="blks")
            nc.sync.dma_start(blks[:], blk_d.ap())
            e1bs = cp.tile([128, B], F32, tag="e1bs")
            nc.sync.dma_start(e1bs[:], e1b_d.ap())
            iotap1 = cp.tile([16, B * TB16], F32, tag="iotap1")
            nc.sync.dma_start(iotap1[:], iotap1_d.ap())
            id4s = cp.tile([B, B], I32, tag="id4s")
            nc.sync.dma_start(id4s[:], id4_d.ap())
            o416s = cp.tile([B, 16], F32, tag="o416s")
            nc.sync.dma_start(o416s[:], o416_d.ap())
            id16s = cp.tile([16, 16], F32, tag="id16s")
            nc.sync.dma_start(id16s[:], id16_d.ap())
            idbfs = cp.tile([128, 128], BF16, tag="idbfs")
            nc.sync.dma_start(idbfs[:], idbf_d.ap())
            o16s = cp.tile([16, 1], F32, tag="o16s")
            nc.sync.dma_start(o16s[:], o16_d.ap())
            mks = cp.tile([CF, 128], F32, tag="mks")
            nc.sync.dma_start(mks[:], mk_d.ap())
            rsels = cp.tile([CF, CS], F32, tag="rsels")
            nc.sync.dma_start(rsels[:], rsel_d.ap())

            # ---------- phase 1: partial scores for my token shard -------
            for nt in range(TSH // 512):
                xt_t = scp.tile([128, DC, 512], F32, tag="xt")
                nc.sync.dma_start(
                    xt_t[:],
                    xt_sh.ap().rearrange("(c p) t -> p c t", p=128)[
                        :, :, nt * 512:(nt + 1) * 512],
                )
                ps_sc = pps.tile([E, 512], F32, tag="sp")
                for dc in range(DC):
                    nc.tensor.matmul(ps_sc[:], lhsT=wg_sb[:, dc, :],
                                     rhs=xt_t[:, dc, :],
                                     start=(dc == 0), stop=(dc == DC - 1))
                sc_sb = scp.tile([E, 512], F32, tag="scsb")
                nc.vector.tensor_copy(sc_sb[:], ps_sc[:])
                nc.sync.dma_start(a2a_in[:, nt * 512:(nt + 1) * 512], sc_sb[:])

            # ---------- phase 2: AllToAll ----------
            nc.gpsimd.collective_compute(
                "AllToAll", ALU.bypass, replica_groups=[list(range(NCORES))],
                ins=[a2a_in.ap()], outs=[a2a_out.ap()],
            )

            PPR = 128 // E
            w128 = cp.tile([128, TPP], F32, tag="w128")
            for r in range(E):
                nc.sync.dma_start(
                    w128[r * PPR:(r + 1) * PPR, :],
                    a2a_out.ap()[r].rearrange("(l f) -> l f", l=PPR))
            w16 = cp.tile([16, B * TB16], F32, tag="w16")
            for r in range(E):
                b, q = divmod(r, RPB)
                nc.sync.dma_start(
                    w16[:, b * TB16 + q * QL: b * TB16 + (q + 1) * QL],
                    a2a_out.ap()[r].rearrange("(s j) -> s j", s=16))

            # ---------- phase 3: softmax pieces ----------
            exp16 = cp.tile([16, B * TB16], F32, tag="exp16")
            parts16 = bp.tile([16, B], F32, tag="parts16")
            for b in range(B):
                sl = slice(b * TB16, (b + 1) * TB16)
                nc.scalar.activation(exp16[:, sl], w16[:, sl], AF.Exp,
                                     accum_out=parts16[:, b:b + 1])
            ps4 = pps.tile([B, 1], F32, tag="sp")
            nc.tensor.matmul(ps4[:], lhsT=parts16[:], rhs=o16s[:],
                             start=True, stop=True)
            recip4 = bp.tile([B, 1], F32, tag="recip4")
            nc.vector.reciprocal(recip4[:], ps4[:])
            diagr = bp.tile([B, B], F32, tag="diagr")
            nc.vector.memset(diagr[:], 0.0)
            nc.vector.copy_predicated(diagr[:], id4s[:],
                                      recip4[:, 0:1].to_broadcast([B, B]))
            psr16 = pps.tile([16, B], F32, tag="sp")
            nc.tensor.matmul(psr16[:], lhsT=o416s[:], rhs=diagr[:],
                             start=True, stop=True)
            recip16 = cp.tile([16, B], F32, tag="recip16")
            nc.vector.tensor_copy(recip16[:], psr16[:])

            # ---------- phase 4: threshold bisection ----------
            lo128 = bp.tile([128, 1], F32, tag="lo128")
            hi128 = bp.tile([128, 1], F32, tag="hi128")
            nc.vector.memset(lo128[:], -cfg.span)
            nc.vector.memset(hi128[:], cfg.span)
            mid128 = bp.tile([128, 1], F32, tag="mid128")
            sel128 = bp.tile([128, 1], I32, tag="sel128")
            seli128 = bp.tile([128, 1], I32, tag="seli128")
            cnt128 = bp.tile([128, 1], F32, tag="cnt128")
            msk = bp.tile([128, TPP], F32, tag="msk")
            for _ in range(cfg.nrounds if cfg.stage >= 1 else 0):
                nc.vector.tensor_add(mid128[:], lo128[:], hi128[:])
                nc.vector.tensor_scalar_mul(mid128[:], mid128[:], 0.5)
                nc.vector.tensor_scalar(msk[:], w128[:], mid128[:, 0:1], None,
                                        op0=ALU.is_ge, op1=ALU.add,
                                        accum_out=cnt128[:, 0:1])
                ptot = pps.tile([128, 1], F32, tag="sp")
                nc.tensor.matmul(ptot[:], lhsT=blks[:], rhs=cnt128[:],
                                 start=True, stop=True)
                nc.vector.tensor_scalar(sel128[:], ptot[:], float(C) - 0.5,
                                        None, op0=ALU.is_ge)
                nc.vector.tensor_scalar(seli128[:], ptot[:], float(C) - 0.5,
                                        None, op0=ALU.is_lt)
                nc.vector.copy_predicated(lo128[:], sel128[:], mid128[:])
                nc.vector.copy_predicated(hi128[:], seli128[:], mid128[:])

            # tau4 / tau16
            ptau = pps.tile([B, 1], F32, tag="sp")
            nc.tensor.matmul(ptau[:], lhsT=e1bs[:], rhs=lo128[:],
                             start=True, stop=True)
            tau4 = bp.tile([B, 1], F32, tag="tau4")
            nc.vector.tensor_copy(tau4[:], ptau[:])
            diagt = bp.tile([B, B], F32, tag="diagt")
            nc.vector.memset(diagt[:], 0.0)
            nc.vector.copy_predicated(diagt[:], id4s[:],
                                      tau4[:, 0:1].to_broadcast([B, B]))
            pst16 = pps.tile([16, B], F32, tag="sp")
            nc.tensor.matmul(pst16[:], lhsT=o416s[:], rhs=diagt[:],
                             start=True, stop=True)
            tau16 = cp.tile([16, B], F32, tag="tau16")
            nc.vector.tensor_copy(tau16[:], pst16[:])

            # ---------- phase 5: compaction + 16->128 relayout (v1) ------
            idxg32s, idxl32s, val128s = [], [], []
            for b in range(B if cfg.stage >= 2 else 0):
                sl = slice(b * TB16, (b + 1) * TB16)
                mask16 = bp.tile([16, TB16], F32, tag="mask16")
                nc.vector.tensor_scalar(mask16[:], w16[:, sl], tau16[:, b:b + 1],
                                        None, op0=ALU.is_ge)
                candi = bp.tile([16, TB16], F32, tag="candi")
                nc.vector.tensor_tensor(candi[:], mask16[:], iotap1[:, sl],
                                        op=ALU.mult)
                nc.vector.tensor_scalar_add(candi[:], candi[:], -1.0)
                candv = bp.tile([16, TB16], F32, tag="candv")
                nc.vector.tensor_tensor(candv[:], mask16[:], exp16[:, sl],
                                        op=ALU.mult)
                nc.vector.tensor_scalar_add(mask16[:], mask16[:], -1.0)
                nc.vector.tensor_tensor(candv[:], candv[:], mask16[:],
                                        op=ALU.add)

                ci = bp.tile([16, CF + 16], F32, tag=f"ci{b}")
                nfi = bp.tile([1, 1], U32, tag=f"nfi{b}")
                nc.gpsimd.sparse_gather(ci[:], candi[:], num_found=nfi[:])
                cv = bp.tile([16, CF + 16], F32, tag=f"cv{b}")
                nfv = bp.tile([1, 1], U32, tag=f"nfv{b}")
                nc.gpsimd.sparse_gather(cv[:], candv[:], num_found=nfv[:])
                nc.sync.dma_start(nf_out.ap()[b:b + 1, 0:1], nfi[:, :])
                nc.sync.dma_start(nf_out.ap()[b:b + 1, 1:2], nfv[:, :])

                nc.vector.tensor_scalar(cv[:, :CF], cv[:, :CF],
                                        recip16[:, b:b + 1], None, op0=ALU.mult)
                nc.vector.tensor_scalar_add(ci[:, :CF], ci[:, :CF],
                                            float(b * T))

                pti = pps.tile([CF, 16], F32, tag="sp")
                nc.tensor.transpose(pti[:], ci[:, :CF], id16s[:])
                cit = bp.tile([CF, 16], F32, tag="cit")
                nc.vector.tensor_copy(cit[:], pti[:])
                ptv = pps.tile([CF, 16], F32, tag="sp")
                nc.tensor.transpose(ptv[:], cv[:, :CF], id16s[:])
                cvt = bp.tile([CF, 16], F32, tag="cvt")
                nc.vector.tensor_copy(cvt[:], ptv[:])

                cmi = bp.tile([CF, 128], F32, tag="cmi")
                nc.vector.tensor_tensor(
                    cmi[:].rearrange("f (g s) -> f g s", g=8),
                    cit[:, None, :].to_broadcast([CF, 8, 16]),
                    mks[:].rearrange("f (g s) -> f g s", g=8),
                    op=ALU.mult)
                cmv = bp.tile([CF, 128], F32, tag="cmv")
                nc.vector.tensor_tensor(
                    cmv[:].rearrange("f (g s) -> f g s", g=8),
                    cvt[:, None, :].to_broadcast([CF, 8, 16]),
                    mks[:].rearrange("f (g s) -> f g s", g=8),
                    op=ALU.mult)

                pri = pps.tile([128, CS], F32, tag="sp")
                nc.tensor.matmul(pri[:], lhsT=cmi[:], rhs=rsels[:],
                                 start=True, stop=True)
                idxg32 = cp.tile([128, CS], I32, name=f"idxg_{b}",
                                 tag=f"idxg_{b}")
                nc.vector.tensor_scalar(idxg32[:], pri[:], 0.0, None,
                                        op0=ALU.max)
                idxl32 = cp.tile([128, CS], I32, name=f"idxl_{b}",
                                 tag=f"idxl_{b}")
                nc.vector.tensor_scalar(idxl32[:], pri[:], float(-b * T),
                                        None, op0=ALU.add)
                prv = pps.tile([128, CS], F32, tag="sp")
                nc.tensor.matmul(prv[:], lhsT=cmv[:], rhs=rsels[:],
                                 start=True, stop=True)
                val128 = cp.tile([128, CS], F32, name=f"val_{b}",
                                 tag=f"val_{b}")
                nc.vector.tensor_copy(val128[:], prv[:])
                idxg32s.append(idxg32)
                idxl32s.append(idxl32)
                val128s.append(val128)

            # ---------- phase 6: per-batch FFN + scatter + inline RS -----
            for b in range(B if cfg.stage >= 3 else 0):
                selTM = fp.tile([128, CS, D], BF16, tag="selTM")
                for cs in range(CS):
                    nc.gpsimd.indirect_dma_start(
                        out=selTM[:, cs, :],
                        out_offset=None,
                        in_=x_bf.ap(),
                        in_offset=IndirectOffsetOnAxis(
                            ap=idxg32s[b][:, cs:cs + 1], axis=0))
                selT = fp.tile([128, DC, C], BF16, tag="selT")
                for cs in range(CS):
                    for dc in range(DC):
                        ptp = pps.tile([128, 128], BF16, tag="tp")
                        nc.tensor.transpose(
                            ptp[:], selTM[:, cs, dc * 128:(dc + 1) * 128],
                            idbfs[:])
                        nc.vector.tensor_copy(
                            selT[:, dc, cs * 128:(cs + 1) * 128], ptp[:])
                pk = pkp.tile([128, CS, ROW], ACC, tag="pk")
                nc.vector.memset(pk[:], 0.0)
                for ct in range(C // NT):
                    csl = slice(ct * NT, (ct + 1) * NT)
                    hT = fp.tile([128, HC, NT], BF16, tag="hT")
                    for ht in range(HC):
                        psh = pmm.tile([128, NT], F32, tag="mm")
                        for dc in range(DC):
                            nc.tensor.matmul(
                                psh[:],
                                lhsT=w1_sb[:, dc, ht * 128:(ht + 1) * 128],
                                rhs=selT[:, dc, csl],
                                start=(dc == 0), stop=(dc == DC - 1))
                        nc.scalar.activation(hT[:, ht, :], psh[:],
                                             getattr(AF, cfg.act))
                    for cl in range(NT // 128):
                        cs = ct * (NT // 128) + cl
                        pso = pmm.tile([128, D], F32, tag="mm")
                        for hc in range(HC):
                            nc.tensor.matmul(
                                pso[:],
                                lhsT=hT[:, hc, cl * 128:(cl + 1) * 128],
                                rhs=w2_sb[:, hc, :],
                                start=(hc == 0), stop=(hc == HC - 1))
                        nc.vector.tensor_scalar(
                            pk[:, cs, :D], pso[:],
                            val128s[b][:, cs:cs + 1], None, op0=ALU.mult)
                        nc.vector.tensor_copy(pk[:, cs, D:D + 1],
                                              val128s[b][:, cs:cs + 1])
                for cs in range(CS):
                    nc.gpsimd.indirect_dma_start(
                        out=dense_b[b].ap(),
                        out_offset=IndirectOffsetOnAxis(
                            ap=idxl32s[b][:, cs:cs + 1], axis=0),
                        in_=pk[:, cs, :],
                        in_offset=None,
                        bounds_check=T - 1,
                        oob_is_err=False)
                if cfg.rs_inline and cfg.stage >= 4:
                    nc.gpsimd.collective_compute(
                        "ReduceScatter", ALU.add,
                        replica_groups=[list(range(NCORES))],
                        ins=[dense_b[b].ap()], outs=[rs_b[b].ap()],
                    )
            if not cfg.rs_inline and cfg.stage >= 4:
                for b in range(B):
                    nc.gpsimd.collective_compute(
                        "ReduceScatter", ALU.add,
                        replica_groups=[list(range(NCORES))],
                        ins=[dense_b[b].ap()], outs=[rs_b[b].ap()],
                    )

            # ---------- phase 7: normalize my shard of each batch --------
            for b in range(B if cfg.stage >= 4 else 0):
                for j in range(TQ // 128):
                    rsl = slice(j * 128, (j + 1) * 128)
                    ld = np_.tile([128, D + 1], ACC, tag="ld")
                    nc.sync.dma_start(ld[:], rs_b[b].ap()[rsl, :D + 1])
                    dn = np_.tile([128, 1], F32, tag="dn")
                    nc.vector.tensor_scalar(dn[:], ld[:, D:D + 1], 1e-8, None,
                                            op0=ALU.max)
                    rc = np_.tile([128, 1], F32, tag="rc")
                    nc.vector.reciprocal(rc[:], dn[:])
                    ot = np_.tile([128, D], F32, tag="ot")
                    nc.vector.tensor_scalar(ot[:], ld[:, :D], rc[:, 0:1], None,
                                            op0=ALU.mult)
                    nc.sync.dma_start(
                        out_sh.ap()[b * TQ + j * 128: b * TQ + (j + 1) * 128, :],
                        ot[:])

    nc.compile()
    return nc


# ---------------------------------------------------------------------------
# host side
# ---------------------------------------------------------------------------

def host_consts(cfg: Cfg = FULL):
    B, T = cfg.B, cfg.T
    TB16, RPB, QL, CF, CS = cfg.TB16, cfg.RPB, cfg.QL, cfg.CF, cfg.CS
    p = np.arange(128)
    blk = (p[:, None] // 32 == p[None, :] // 32).astype(np.float32)
    e1b = (p[:, None] // 32 == np.arange(B)[None, :]).astype(np.float32) / 32.0
    iotap1 = np.zeros((16, B * TB16), np.float32)
    for s in range(16):
        for q in range(RPB):
            j = np.arange(QL)
            t = q * (T // RPB) + s * QL + j
            for b in range(B):
                iotap1[s, b * TB16 + q * QL + j] = t + 1
    id4 = np.eye(B, dtype=np.int32)
    o416 = np.ones((B, 16), np.float32)
    id16 = np.eye(16, dtype=np.float32)
    idbf = np.eye(128).astype(ml_dtypes.bfloat16)
    o16 = np.ones((16, 1), np.float32)
    f = np.arange(CF)
    g = np.arange(8)
    mk = np.zeros((CF, 128), np.float32)
    mk.reshape(CF, 8, 16)[:, :, :] = (f[:, None] % 8 == g[None, :]).astype(
        np.float32)[:, :, None]
    rsel = (f[:, None] // 8 == np.arange(CS)[None, :]).astype(np.float32)
    return dict(blk128=blk, e1b=e1b, iotap1=iotap1, id4=id4, o416=o416,
                id16=id16, idbf=idbf, o16=o16, mk=mk, rsel=rsel)


def make_in_maps(inputs, cfg: Cfg = FULL):
    x = np.asarray(inputs["x"], np.float32).reshape(cfg.BT, cfg.D)
    Wg = np.ascontiguousarray(np.asarray(inputs["Wg"], np.float32))
    W1 = np.asarray(inputs["W1"], np.float32)
    W2 = np.asarray(inputs["W2"], np.float32)
    consts = host_consts(cfg)
    x_bf = x.astype(ml_dtypes.bfloat16)
    in_maps = []
    for i in range(NCORES):
        m = dict(consts)
        m["x_bf"] = x_bf
        m["xt_sh"] = np.ascontiguousarray(x[i * cfg.TSH:(i + 1) * cfg.TSH].T)
        m["wg"] = Wg
        m["w1"] = np.ascontiguousarray(W1[i].astype(ml_dtypes.bfloat16))
        m["w2"] = np.ascontiguousarray(W2[i].astype(ml_dtypes.bfloat16))
        in_maps.append(m)
    return in_maps


def assemble_out(results, cfg: Cfg = FULL):
    nf = np.stack([np.asarray(results[i]["nf_out"]) for i in range(NCORES)])
    if not (nf == cfg.C).all():
        print(f"WARNING: sparse_gather num_found != {cfg.C}: {nf.tolist()}",
              file=sys.stderr)
    TQ = 1024
    out = np.empty((cfg.B, cfg.T, cfg.D), np.float32)
    for i in range(NCORES):
        r = np.asarray(results[i]["out_sh"]).reshape(cfg.B, TQ, cfg.D)
        for b in range(cfg.B):
            out[b, i * TQ:(i + 1) * TQ] = r[b]
    return out


_NC_CACHE = {}


def get_nc():
    if "nc" not in _NC_CACHE:
        _NC_CACHE["nc"] = build_nc(_NC_CACHE.get("cfg", FULL))
    return _NC_CACHE["nc"]


def kernel(**inputs):
    nc = get_nc()
    in_maps = make_in_maps(inputs, _NC_CACHE.get("cfg", FULL))
    res = run_bass_kernel_spmd(nc, in_maps, core_ids=list(range(NCORES)),
                               **_NC_CACHE.get("run_kwargs", {}))
    _NC_CACHE["last_run"] = res
    return assemble_out(res.results, FULL)


# revision 70
# speedup vs baseline: 1.0160x; 1.0160x over previous
"""Expert-choice MoE router kernel for Trainium2 (8 NeuronCores), v3.

Problem (B=4, T=8192, D=512, E=8, H=2048, C=1024):
  scores = x @ Wg; w = softmax over T per (b,e); top-C tokens per (b,e);
  y = gelu(x[sel] @ W1) @ W2 * w[sel]; out = scatter_add(y)/max(sum w, 1e-8)

Sharding: expert-parallel, one expert per core.

v3 = v1's proven selection/compaction machinery + the v2 wins that were
individually validated on HW:
  - dense accumulator: bf16, split per batch (4 tensors), zeroed on the
    scalar-engine DMA queue (v1 lost 200 us blocking the sync queue).
  - ReduceScatter: bf16, per batch, issued inside the FFN loop so RS(b)
    overlaps FFN(b+1).
  - bisection: 28 rounds over [-8, 8] (max|score| ~4.9, top-C gap 3.7e-6
    >> 16/2^28), one PE trip per round via block-diag segment-sum.
  - compaction: v1 verbatim (two sparse_gathers per batch on (16, 512)
    slices + selection-matmul relayout) -- the v2 paired variant crashes
    the exec unit on HW.
"""

import sys
from dataclasses import dataclass

sys.path.insert(0, "/opt/trn_rl_repo")

import numpy as np
import ml_dtypes

import concourse.bass as bass  # noqa: F401
import concourse.mybir as mybir
import concourse.tile as tile
from concourse import bacc
from concourse.bass import IndirectOffsetOnAxis
from concourse.bass_utils import run_bass_kernel_spmd

F32 = mybir.dt.float32
BF16 = mybir.dt.bfloat16
I32 = mybir.dt.int32
U32 = mybir.dt.uint32
AF = mybir.ActivationFunctionType
ALU = mybir.AluOpType

NCORES = 8


@dataclass(frozen=True)
class Cfg:
    B: int = 4
    T: int = 8192
    D: int = 512
    E: int = 8
    H: int = 2048
    C: int = 1024
    nrounds: int = 28
    span: float = 8.0
    act: str = "Gelu"
    acc_bf16: bool = True
    rs_inline: bool = True
    stage: int = 4   # 1=+bisect, 2=+selection, 3=+FFN, 4=full

    @property
    def BT(self):
        return self.B * self.T

    @property
    def TSH(self):
        return self.BT // NCORES

    @property
    def ROW(self):
        return self.D + 8

    @property
    def DC(self):
        return self.D // 128

    @property
    def HC(self):
        return self.H // 128

    @property
    def TPP(self):
        return self.T * self.B // 128

    @property
    def RPB(self):
        return NCORES // self.B

    @property
    def QL(self):
        return self.T // self.RPB // 16

    @property
    def TB16(self):
        return self.T // 16

    @property
    def CF(self):
        return self.C // 16

    @property
    def CS(self):
        return self.C // 128


FULL = Cfg()


def build_nc(cfg: Cfg = FULL):
    B, T, D, E, H, C = cfg.B, cfg.T, cfg.D, cfg.E, cfg.H, cfg.C
    BT, TSH, ROW, DC, HC = cfg.BT, cfg.TSH, cfg.ROW, cfg.DC, cfg.HC
    TPP, RPB, QL, TB16 = cfg.TPP, cfg.RPB, cfg.QL, cfg.TB16
    CF, CS = cfg.CF, cfg.CS
    NT = 512
    TQ = 1024               # rows per core of a per-batch ReduceScatter
    ACC = BF16 if cfg.acc_bf16 else F32

    nc = bacc.Bacc("TRN2", target_bir_lowering=False, debug=False,
                   num_devices=NCORES)

    # ---- I/O ----
    x_bf = nc.dram_tensor("x_bf", [BT, D], BF16, kind="ExternalInput")
    xt_sh = nc.dram_tensor("xt_sh", [D, TSH], F32, kind="ExternalInput")
    wg_d = nc.dram_tensor("wg", [D, E], F32, kind="ExternalInput")
    w1_d = nc.dram_tensor("w1", [D, H], BF16, kind="ExternalInput")
    w2_d = nc.dram_tensor("w2", [H, D], BF16, kind="ExternalInput")
    blk_d = nc.dram_tensor("blk128", [128, 128], F32, kind="ExternalInput")
    e1b_d = nc.dram_tensor("e1b", [128, B], F32, kind="ExternalInput")
    iotap1_d = nc.dram_tensor("iotap1", [16, B * TB16], F32,
                              kind="ExternalInput")
    id4_d = nc.dram_tensor("id4", [B, B], I32, kind="ExternalInput")
    o416_d = nc.dram_tensor("o416", [B, 16], F32, kind="ExternalInput")
    id16_d = nc.dram_tensor("id16", [16, 16], F32, kind="ExternalInput")
    idbf_d = nc.dram_tensor("idbf", [128, 128], BF16, kind="ExternalInput")
    o16_d = nc.dram_tensor("o16", [16, 1], F32, kind="ExternalInput")
    mk_d = nc.dram_tensor("mk", [CF, 128], F32, kind="ExternalInput")
    rsel_d = nc.dram_tensor("rsel", [CF, CS], F32, kind="ExternalInput")

    out_sh = nc.dram_tensor("out_sh", [TSH, D], F32, kind="ExternalOutput")
    nf_out = nc.dram_tensor("nf_out", [B, 2], U32, kind="ExternalOutput")

    # ---- internal DRAM ----
    a2a_in = nc.dram_tensor("a2a_in", [E, TSH], F32)
    a2a_out = nc.dram_tensor("a2a_out", [E, TSH], F32)
    dense_b = [nc.dram_tensor(f"dense{b}", [T, ROW], ACC) for b in range(B)]
    rs_b = [nc.dram_tensor(f"rs{b}", [TQ, ROW], ACC) for b in range(B)]

    with tile.TileContext(nc) as tc:
        with (
            tc.tile_pool(name="const", bufs=1) as cp,
            tc.tile_pool(name="sc", bufs=2) as scp,
            tc.tile_pool(name="bis", bufs=1) as bp,
            tc.tile_pool(name="ffn", bufs=2) as fp,
            tc.tile_pool(name="pk", bufs=2 if cfg.acc_bf16 else 1) as pkp,
            tc.tile_pool(name="norm", bufs=2) as np_,
            tc.tile_pool(name="pmm", bufs=2, space="PSUM") as pmm,
            tc.tile_pool(name="pps", bufs=3, space="PSUM") as pps,
        ):
            # ---------- phase 0: zero dense accumulators (scalar queue) ---
            ZR = 8 if cfg.acc_bf16 else 4
            zt = cp.tile([128, ZR * ROW], ACC, tag="zt")
            nc.vector.memset(zt[:], 0.0)
            for b in range(B):
                dz = dense_b[b].ap().rearrange(
                    "(j p zr) r -> j p (zr r)", p=128, zr=ZR)
                for j in range(T // (128 * ZR)):
                    nc.scalar.dma_start(dz[j], zt[:])

            # ---------- load constants / weights ----------
            wg_sb = cp.tile([128, DC, E], F32, tag="wg_sb")
            nc.sync.dma_start(wg_sb[:], wg_d.ap().rearrange("(c p) e -> p c e", p=128))
            w1_sb = cp.tile([128, DC, H], BF16, tag="w1_sb")
            nc.sync.dma_start(w1_sb[:], w1_d.ap().rearrange("(c p) h -> p c h", p=128))
            w2_sb = cp.tile([128, HC, D], BF16, tag="w2_sb")
            nc.sync.dma_start(w2_sb[:], w2_d.ap().rearrange("(c p) d -> p c d", p=128))
            blks = cp.tile([128, 128], F32, tag="blks")
            nc.sync.dma_start(blks[:], blk_d.ap())
            e1bs = cp.tile([128, B], F32, tag="e1bs")
            nc.sync.dma_start(e1bs[:], e1b_d.ap())
            iotap1 = cp.tile([16, B * TB16], F32, tag="iotap1")
            nc.sync.dma_start(iotap1[:], iotap1_d.ap())
            id4s = cp.tile([B, B], I32, tag="id4s")
            nc.sync.dma_start(id4s[:], id4_d.ap())
            o416s = cp.tile([B, 16], F32, tag="o416s")
            nc.sync.dma_start(o416s[:], o416_d.ap())
            id16s = cp.tile([16, 16], F32, tag="id16s")
            nc.sync.dma_start(id16s[:], id16_d.ap())
            idbfs = cp.tile([128, 128], BF16, tag="idbfs")
            nc.sync.dma_start(idbfs[:], idbf_d.ap())
            o16s = cp.tile([16, 1], F32, tag="o16s")
            nc.sync.dma_start(o16s[:], o16_d.ap())
            mks = cp.tile([CF, 128], F32, tag="mks")
            nc.sync.dma_start(mks[:], mk_d.ap())
            rsels = cp.tile([CF, CS], F32, tag="rsels")
            nc.sync.dma_start(rsels[:], rsel_d.ap())

            # ---------- phase 1: partial scores for my token shard -------
            for nt in range(TSH // 512):
                xt_t = scp.tile([128, DC, 512], F32, tag="xt")
                nc.sync.dma_start(
                    xt_t[:],
                    xt_sh.ap().rearrange("(c p) t -> p c t", p=128)[
                        :, :, nt * 512:(nt + 1) * 512],
                )
                ps_sc = pps.tile([E, 512], F32, tag="sp")
                for dc in range(DC):
                    nc.tensor.matmul(ps_sc[:], lhsT=wg_sb[:, dc, :],
                                     rhs=xt_t[:, dc, :],
                                     start=(dc == 0), stop=(dc == DC - 1))
                sc_sb = scp.tile([E, 512], F32, tag="scsb")
                nc.vector.tensor_copy(sc_sb[:], ps_sc[:])
                nc.sync.dma_start(a2a_in[:, nt * 512:(nt + 1) * 512], sc_sb[:])

            # ---------- phase 2: AllToAll ----------
            nc.gpsimd.collective_compute(
                "AllToAll", ALU.bypass, replica_groups=[list(range(NCORES))],
                ins=[a2a_in.ap()], outs=[a2a_out.ap()],
            )

            PPR = 128 // E
            w128 = cp.tile([128, TPP], F32, tag="w128")
            for r in range(E):
                nc.sync.dma_start(
                    w128[r * PPR:(r + 1) * PPR, :],
                    a2a_out.ap()[r].rearrange("(l f) -> l f", l=PPR))
            w16 = cp.tile([16, B * TB16], F32, tag="w16")
            for r in range(E):
                b, q = divmod(r, RPB)
                nc.sync.dma_start(
                    w16[:, b * TB16 + q * QL: b * TB16 + (q + 1) * QL],
                    a2a_out.ap()[r].rearrange("(s j) -> s j", s=16))

            # ---------- phase 3: softmax pieces ----------
            exp16 = cp.tile([16, B * TB16], F32, tag="exp16")
            parts16 = bp.tile([16, B], F32, tag="parts16")
            for b in range(B):
                sl = slice(b * TB16, (b + 1) * TB16)
                nc.scalar.activation(exp16[:, sl], w16[:, sl], AF.Exp,
                                     accum_out=parts16[:, b:b + 1])
            ps4 = pps.tile([B, 1], F32, tag="sp")
            nc.tensor.matmul(ps4[:], lhsT=parts16[:], rhs=o16s[:],
                             start=True, stop=True)
            recip4 = bp.tile([B, 1], F32, tag="recip4")
            nc.vector.reciprocal(recip4[:], ps4[:])
            diagr = bp.tile([B, B], F32, tag="diagr")
            nc.vector.memset(diagr[:], 0.0)
            nc.vector.copy_predicated(diagr[:], id4s[:],
                                      recip4[:, 0:1].to_broadcast([B, B]))
            psr16 = pps.tile([16, B], F32, tag="sp")
            nc.tensor.matmul(psr16[:], lhsT=o416s[:], rhs=diagr[:],
                             start=True, stop=True)
            recip16 = cp.tile([16, B], F32, tag="recip16")
            nc.vector.tensor_copy(recip16[:], psr16[:])

            # ---------- phase 4: threshold bisection ----------
            lo128 = bp.tile([128, 1], F32, tag="lo128")
            hi128 = bp.tile([128, 1], F32, tag="hi128")
            nc.vector.memset(lo128[:], -cfg.span)
            nc.vector.memset(hi128[:], cfg.span)
            mid128 = bp.tile([128, 1], F32, tag="mid128")
            sel128 = bp.tile([128, 1], I32, tag="sel128")
            seli128 = bp.tile([128, 1], I32, tag="seli128")
            cnt128 = bp.tile([128, 1], F32, tag="cnt128")
            msk = bp.tile([128, TPP], F32, tag="msk")
            for _ in range(cfg.nrounds if cfg.stage >= 1 else 0):
                nc.vector.tensor_add(mid128[:], lo128[:], hi128[:])
                nc.vector.tensor_scalar_mul(mid128[:], mid128[:], 0.5)
                nc.vector.tensor_scalar(msk[:], w128[:], mid128[:, 0:1], None,
                                        op0=ALU.is_ge, op1=ALU.add,
                                        accum_out=cnt128[:, 0:1])
                ptot = pps.tile([128, 1], F32, tag="sp")
                nc.tensor.matmul(ptot[:], lhsT=blks[:], rhs=cnt128[:],
                                 start=True, stop=True)
                nc.vector.tensor_scalar(sel128[:], ptot[:], float(C) - 0.5,
                                        None, op0=ALU.is_ge)
                nc.vector.tensor_scalar(seli128[:], ptot[:], float(C) - 0.5,
                                        None, op0=ALU.is_lt)
                nc.vector.copy_predicated(lo128[:], sel128[:], mid128[:])
                nc.vector.copy_predicated(hi128[:], seli128[:], mid128[:])

            # tau4 / tau16
            ptau = pps.tile([B, 1], F32, tag="sp")
            nc.tensor.matmul(ptau[:], lhsT=e1bs[:], rhs=lo128[:],
                             start=True, stop=True)
            tau4 = bp.tile([B, 1], F32, tag="tau4")
            nc.vector.tensor_copy(tau4[:], ptau[:])
            diagt = bp.tile([B, B], F32, tag="diagt")
            nc.vector.memset(diagt[:], 0.0)
            nc.vector.copy_predicated(diagt[:], id4s[:],
                                      tau4[:, 0:1].to_broadcast([B, B]))
            pst16 = pps.tile([16, B], F32, tag="sp")
            nc.tensor.matmul(pst16[:], lhsT=o416s[:], rhs=diagt[:],
                             start=True, stop=True)
            tau16 = cp.tile([16, B], F32, tag="tau16")
            nc.vector.tensor_copy(tau16[:], pst16[:])

            # ---------- phase 5: compaction + 16->128 relayout (v1) ------
            idxg32s, idxl32s, val128s, selTs = [], [], [], []
            for b in range(B if cfg.stage >= 2 else 0):
                sl = slice(b * TB16, (b + 1) * TB16)
                mask16 = bp.tile([16, TB16], F32, tag="mask16")
                nc.vector.tensor_scalar(mask16[:], w16[:, sl], tau16[:, b:b + 1],
                                        None, op0=ALU.is_ge)
                candi = bp.tile([16, TB16], F32, tag="candi")
                nc.vector.tensor_tensor(candi[:], mask16[:], iotap1[:, sl],
                                        op=ALU.mult)
                nc.vector.tensor_scalar_add(candi[:], candi[:], -1.0)
                candv = bp.tile([16, TB16], F32, tag="candv")
                nc.vector.tensor_tensor(candv[:], mask16[:], exp16[:, sl],
                                        op=ALU.mult)
                nc.vector.tensor_scalar_add(mask16[:], mask16[:], -1.0)
                nc.vector.tensor_tensor(candv[:], candv[:], mask16[:],
                                        op=ALU.add)

                ci = bp.tile([16, CF + 16], F32, tag=f"ci{b}")
                nfi = bp.tile([1, 1], U32, tag=f"nfi{b}")
                nc.gpsimd.sparse_gather(ci[:], candi[:], num_found=nfi[:])
                cv = bp.tile([16, CF + 16], F32, tag=f"cv{b}")
                nfv = bp.tile([1, 1], U32, tag=f"nfv{b}")
                nc.gpsimd.sparse_gather(cv[:], candv[:], num_found=nfv[:])
                nc.sync.dma_start(nf_out.ap()[b:b + 1, 0:1], nfi[:, :])
                nc.sync.dma_start(nf_out.ap()[b:b + 1, 1:2], nfv[:, :])

                nc.vector.tensor_scalar(cv[:, :CF], cv[:, :CF],
                                        recip16[:, b:b + 1], None, op0=ALU.mult)
                nc.vector.tensor_scalar_add(ci[:, :CF], ci[:, :CF],
                                            float(b * T))

                pti = pps.tile([CF, 16], F32, tag="sp")
                nc.tensor.transpose(pti[:], ci[:, :CF], id16s[:])
                cit = bp.tile([CF, 16], F32, tag="cit")
                nc.vector.tensor_copy(cit[:], pti[:])
                ptv = pps.tile([CF, 16], F32, tag="sp")
                nc.tensor.transpose(ptv[:], cv[:, :CF], id16s[:])
                cvt = bp.tile([CF, 16], F32, tag="cvt")
                nc.vector.tensor_copy(cvt[:], ptv[:])

                cmi = bp.tile([CF, 128], F32, tag="cmi")
                nc.vector.tensor_tensor(
                    cmi[:].rearrange("f (g s) -> f g s", g=8),
                    cit[:, None, :].to_broadcast([CF, 8, 16]),
                    mks[:].rearrange("f (g s) -> f g s", g=8),
                    op=ALU.mult)
                cmv = bp.tile([CF, 128], F32, tag="cmv")
                nc.vector.tensor_tensor(
                    cmv[:].rearrange("f (g s) -> f g s", g=8),
                    cvt[:, None, :].to_broadcast([CF, 8, 16]),
                    mks[:].rearrange("f (g s) -> f g s", g=8),
                    op=ALU.mult)

                pri = pps.tile([128, CS], F32, tag="sp")
                nc.tensor.matmul(pri[:], lhsT=cmi[:], rhs=rsels[:],
                                 start=True, stop=True)
                idxg32 = cp.tile([128, CS], I32, name=f"idxg_{b}",
                                 tag=f"idxg_{b}")
                nc.vector.tensor_scalar(idxg32[:], pri[:], 0.0, None,
                                        op0=ALU.max)
                idxl32 = cp.tile([128, CS], I32, name=f"idxl_{b}",
                                 tag=f"idxl_{b}")
                nc.vector.tensor_scalar(idxl32[:], pri[:], float(-b * T),
                                        None, op0=ALU.add)
                prv = pps.tile([128, CS], F32, tag="sp")
                nc.tensor.matmul(prv[:], lhsT=cmv[:], rhs=rsels[:],
                                 start=True, stop=True)
                val128 = cp.tile([128, CS], F32, name=f"val_{b}",
                                 tag=f"val_{b}")
                nc.vector.tensor_copy(val128[:], prv[:])
                idxg32s.append(idxg32)
                idxl32s.append(idxl32)
                val128s.append(val128)

                # gather + transpose now, BEFORE any ReduceScatter is in
                # flight: PE transposes are serialized against collectives
                # by the tile framework, which stalled the FFN ~50us/batch.
                if cfg.stage >= 3:
                    selTM = fp.tile([128, CS, D], BF16, tag="selTM")
                    for cs in range(CS):
                        nc.gpsimd.indirect_dma_start(
                            out=selTM[:, cs, :],
                            out_offset=None,
                            in_=x_bf.ap(),
                            in_offset=IndirectOffsetOnAxis(
                                ap=idxg32[:, cs:cs + 1], axis=0))
                    selT = cp.tile([128, DC, C], BF16, name=f"selT_{b}",
                                   tag=f"selT_{b}")
                    for cs in range(CS):
                        for dc in range(DC):
                            ptp = pps.tile([128, 128], BF16, tag="tp")
                            nc.tensor.transpose(
                                ptp[:], selTM[:, cs, dc * 128:(dc + 1) * 128],
                                idbfs[:])
                            nc.vector.tensor_copy(
                                selT[:, dc, cs * 128:(cs + 1) * 128], ptp[:])
                    selTs.append(selT)

            # ---------- phase 6: per-batch FFN + scatter + inline RS -----
            for b in range(B if cfg.stage >= 3 else 0):
                selT = selTs[b]
                pk = pkp.tile([128, CS, ROW], ACC, tag="pk")
                nc.vector.memset(pk[:], 0.0)
                for ct in range(C // NT):
                    csl = slice(ct * NT, (ct + 1) * NT)
                    hT = fp.tile([128, HC, NT], BF16, tag="hT")
                    for ht in range(HC):
                        psh = pmm.tile([128, NT], F32, tag="mm")
                        for dc in range(DC):
                            nc.tensor.matmul(
                                psh[:],
                                lhsT=w1_sb[:, dc, ht * 128:(ht + 1) * 128],
                                rhs=selT[:, dc, csl],
                                start=(dc == 0), stop=(dc == DC - 1))
                        nc.scalar.activation(hT[:, ht, :], psh[:],
                                             getattr(AF, cfg.act))
                    for cl in range(NT // 128):
                        cs = ct * (NT // 128) + cl
                        pso = pmm.tile([128, D], F32, tag="mm")
                        for hc in range(HC):
                            nc.tensor.matmul(
                                pso[:],
                                lhsT=hT[:, hc, cl * 128:(cl + 1) * 128],
                                rhs=w2_sb[:, hc, :],
                                start=(hc == 0), stop=(hc == HC - 1))
                        nc.vector.tensor_scalar(
                            pk[:, cs, :D], pso[:],
                            val128s[b][:, cs:cs + 1], None, op0=ALU.mult)
                        nc.vector.tensor_copy(pk[:, cs, D:D + 1],
                                              val128s[b][:, cs:cs + 1])
                for cs in range(CS):
                    nc.gpsimd.indirect_dma_start(
                        out=dense_b[b].ap(),
                        out_offset=IndirectOffsetOnAxis(
                            ap=idxl32s[b][:, cs:cs + 1], axis=0),
                        in_=pk[:, cs, :],
                        in_offset=None,
                        bounds_check=T - 1,
                        oob_is_err=False)
                if cfg.rs_inline and cfg.stage >= 4:
                    nc.gpsimd.collective_compute(
                        "ReduceScatter", ALU.add,
                        replica_groups=[list(range(NCORES))],
                        ins=[dense_b[b].ap()], outs=[rs_b[b].ap()],
                    )
            if not cfg.rs_inline and cfg.stage >= 4:
                for b in range(B):
                    nc.gpsimd.collective_compute(
                        "ReduceScatter", ALU.add,
                        replica_groups=[list(range(NCORES))],
                        ins=[dense_b[b].ap()], outs=[rs_b[b].ap()],
                    )

            # ---------- phase 7: normalize my shard of each batch --------
            for b in range(B if cfg.stage >= 4 else 0):
                for j in range(TQ // 128):
                    rsl = slice(j * 128, (j + 1) * 128)
                    ld = np_.tile([128, D + 1], ACC, tag="ld")
                    nc.sync.dma_start(ld[:], rs_b[b].ap()[rsl, :D + 1])
                    dn = np_.tile([128, 1], F32, tag="dn")
                    nc.vector.tensor_scalar(dn[:], ld[:, D:D + 1], 1e-8, None,
                                            op0=ALU.max)
                    rc = np_.tile([128, 1], F32, tag="rc")
                    nc.vector.reciprocal(rc[:], dn[:])
                    ot = np_.tile([128, D], F32, tag="ot")
                    nc.vector.tensor_scalar(ot[:], ld[:, :D], rc[:, 0:1], None,
                                            op0=ALU.mult)
                    nc.sync.dma_start(
                        out_sh.ap()[b * TQ + j * 128: b * TQ + (j + 1) * 128, :],
                        ot[:])

    nc.compile()
    return nc


# ---------------------------------------------------------------------------
# host side
# ---------------------------------------------------------------------------

def host_consts(cfg: Cfg = FULL):
    B, T = cfg.B, cfg.T
    TB16, RPB, QL, CF, CS = cfg.TB16, cfg.RPB, cfg.QL, cfg.CF, cfg.CS
    p = np.arange(128)
    blk = (p[:, None] // 32 == p[None, :] // 32).astype(np.float32)
    e1b = (p[:, None] // 32 == np.arange(B)[None, :]).astype(np.float32) / 32.0
    iotap1 = np.zeros((16, B * TB16), np.float32)
    for s in range(16):
        for q in range(RPB):
            j = np.arange(QL)
            t = q * (T // RPB) + s * QL + j
            for b in range(B):
                iotap1[s, b * TB16 + q * QL + j] = t + 1
    id4 = np.eye(B, dtype=np.int32)
    o416 = np.ones((B, 16), np.float32)
    id16 = np.eye(16, dtype=np.float32)
    idbf = np.eye(128).astype(ml_dtypes.bfloat16)
    o16 = np.ones((16, 1), np.float32)
    f = np.arange(CF)
    g = np.arange(8)
    mk = np.zeros((CF, 128), np.float32)
    mk.reshape(CF, 8, 16)[:, :, :] = (f[:, None] % 8 == g[None, :]).astype(
        np.float32)[:, :, None]
    rsel = (f[:, None] // 8 == np.arange(CS)[None, :]).astype(np.float32)
    return dict(blk128=blk, e1b=e1b, iotap1=iotap1, id4=id4, o416=o416,
                id16=id16, idbf=idbf, o16=o16, mk=mk, rsel=rsel)


def make_in_maps(inputs, cfg: Cfg = FULL):
    x = np.asarray(inputs["x"], np.float32).reshape(cfg.BT, cfg.D)
    Wg = np.ascontiguousarray(np.asarray(inputs["Wg"], np.float32))
    W1 = np.asarray(inputs["W1"], np.float32)
    W2 = np.asarray(inputs["W2"], np.float32)
    consts = host_consts(cfg)
    x_bf = x.astype(ml_dtypes.bfloat16)
    in_maps = []
    for i in range(NCORES):
        m = dict(consts)
        m["x_bf"] = x_bf
        m["xt_sh"] = np.ascontiguousarray(x[i * cfg.TSH:(i + 1) * cfg.TSH].T)
        m["wg"] = Wg
        m["w1"] = np.ascontiguousarray(W1[i].astype(ml_dtypes.bfloat16))
        m["w2"] = np.ascontiguousarray(W2[i].astype(ml_dtypes.bfloat16))
        in_maps.append(m)
    return in_maps


def assemble_out(results, cfg: Cfg = FULL):
    nf = np.stack([np.asarray(results[i]["nf_out"]) for i in range(NCORES)])
    if not (nf == cfg.C).all():
        print(f"WARNING: sparse_gather num_found != {cfg.C}: {nf.tolist()}",
              file=sys.stderr)
    TQ = 1024
    out = np.empty((cfg.B, cfg.T, cfg.D), np.float32)
    for i in range(NCORES):
        r = np.asarray(results[i]["out_sh"]).reshape(cfg.B, TQ, cfg.D)
        for b in range(cfg.B):
            out[b, i * TQ:(i + 1) * TQ] = r[b]
    return out


_NC_CACHE = {}


def get_nc():
    if "nc" not in _NC_CACHE:
        _NC_CACHE["nc"] = build_nc(_NC_CACHE.get("cfg", FULL))
    return _NC_CACHE["nc"]


def kernel(**inputs):
    nc = get_nc()
    in_maps = make_in_maps(inputs, _NC_CACHE.get("cfg", FULL))
    res = run_bass_kernel_spmd(nc, in_maps, core_ids=list(range(NCORES)),
                               **_NC_CACHE.get("run_kwargs", {}))
    _NC_CACHE["last_run"] = res
    return assemble_out(res.results, FULL)


# revision 71
# speedup vs baseline: 1.0268x; 1.0106x over previous
"""Expert-choice MoE router kernel for Trainium2 (8 NeuronCores), v3.

Problem (B=4, T=8192, D=512, E=8, H=2048, C=1024):
  scores = x @ Wg; w = softmax over T per (b,e); top-C tokens per (b,e);
  y = gelu(x[sel] @ W1) @ W2 * w[sel]; out = scatter_add(y)/max(sum w, 1e-8)

Sharding: expert-parallel, one expert per core.

v3 = v1's proven selection/compaction machinery + the v2 wins that were
individually validated on HW:
  - dense accumulator: bf16, split per batch (4 tensors), zeroed on the
    scalar-engine DMA queue (v1 lost 200 us blocking the sync queue).
  - ReduceScatter: bf16, per batch, issued inside the FFN loop so RS(b)
    overlaps FFN(b+1).
  - bisection: 28 rounds over [-8, 8] (max|score| ~4.9, top-C gap 3.7e-6
    >> 16/2^28), one PE trip per round via block-diag segment-sum.
  - compaction: v1 verbatim (two sparse_gathers per batch on (16, 512)
    slices + selection-matmul relayout) -- the v2 paired variant crashes
    the exec unit on HW.
"""

import sys
from dataclasses import dataclass

sys.path.insert(0, "/opt/trn_rl_repo")

import numpy as np
import ml_dtypes

import concourse.bass as bass  # noqa: F401
import concourse.mybir as mybir
import concourse.tile as tile
from concourse import bacc
from concourse.bass import IndirectOffsetOnAxis
from concourse.bass_utils import run_bass_kernel_spmd

F32 = mybir.dt.float32
BF16 = mybir.dt.bfloat16
I32 = mybir.dt.int32
U32 = mybir.dt.uint32
AF = mybir.ActivationFunctionType
ALU = mybir.AluOpType

NCORES = 8


@dataclass(frozen=True)
class Cfg:
    B: int = 4
    T: int = 8192
    D: int = 512
    E: int = 8
    H: int = 2048
    C: int = 1024
    nrounds: int = 28
    span: float = 8.0
    act: str = "Gelu"
    acc_bf16: bool = True
    rs_inline: bool = True
    stage: int = 4   # 1=+bisect, 2=+selection, 3=+FFN, 4=full

    @property
    def BT(self):
        return self.B * self.T

    @property
    def TSH(self):
        return self.BT // NCORES

    @property
    def ROW(self):
        return self.D + 8

    @property
    def DC(self):
        return self.D // 128

    @property
    def HC(self):
        return self.H // 128

    @property
    def TPP(self):
        return self.T * self.B // 128

    @property
    def RPB(self):
        return NCORES // self.B

    @property
    def QL(self):
        return self.T // self.RPB // 16

    @property
    def TB16(self):
        return self.T // 16

    @property
    def CF(self):
        return self.C // 16

    @property
    def CS(self):
        return self.C // 128


FULL = Cfg()


def build_nc(cfg: Cfg = FULL):
    B, T, D, E, H, C = cfg.B, cfg.T, cfg.D, cfg.E, cfg.H, cfg.C
    BT, TSH, ROW, DC, HC = cfg.BT, cfg.TSH, cfg.ROW, cfg.DC, cfg.HC
    TPP, RPB, QL, TB16 = cfg.TPP, cfg.RPB, cfg.QL, cfg.TB16
    CF, CS = cfg.CF, cfg.CS
    NT = 512
    TQ = 1024               # rows per core of a per-batch ReduceScatter
    ACC = BF16 if cfg.acc_bf16 else F32

    nc = bacc.Bacc("TRN2", target_bir_lowering=False, debug=False,
                   num_devices=NCORES)

    # ---- I/O ----
    x_bf = nc.dram_tensor("x_bf", [BT, D], BF16, kind="ExternalInput")
    xt_sh = nc.dram_tensor("xt_sh", [D, TSH], F32, kind="ExternalInput")
    wg_d = nc.dram_tensor("wg", [D, E], F32, kind="ExternalInput")
    w1_d = nc.dram_tensor("w1", [D, H], BF16, kind="ExternalInput")
    w2_d = nc.dram_tensor("w2", [H, D], BF16, kind="ExternalInput")
    blk_d = nc.dram_tensor("blk128", [128, 128], F32, kind="ExternalInput")
    e1b_d = nc.dram_tensor("e1b", [128, B], F32, kind="ExternalInput")
    iotap1_d = nc.dram_tensor("iotap1", [16, B * TB16], F32,
                              kind="ExternalInput")
    id4_d = nc.dram_tensor("id4", [B, B], I32, kind="ExternalInput")
    o416_d = nc.dram_tensor("o416", [B, 16], F32, kind="ExternalInput")
    id16_d = nc.dram_tensor("id16", [16, 16], F32, kind="ExternalInput")
    idbf_d = nc.dram_tensor("idbf", [128, 128], BF16, kind="ExternalInput")
    o16_d = nc.dram_tensor("o16", [16, 1], F32, kind="ExternalInput")
    mk_d = nc.dram_tensor("mk", [CF, 128], F32, kind="ExternalInput")
    rsel_d = nc.dram_tensor("rsel", [CF, CS], F32, kind="ExternalInput")

    out_sh = nc.dram_tensor("out_sh", [TSH, D], F32, kind="ExternalOutput")
    nf_out = nc.dram_tensor("nf_out", [B, 2], U32, kind="ExternalOutput")

    # ---- internal DRAM ----
    a2a_in = nc.dram_tensor("a2a_in", [E, TSH], F32)
    a2a_out = nc.dram_tensor("a2a_out", [E, TSH], F32)
    dense_b = [nc.dram_tensor(f"dense{b}", [T, ROW], ACC) for b in range(B)]
    rs_b = [nc.dram_tensor(f"rs{b}", [TQ, ROW], ACC) for b in range(B)]

    with tile.TileContext(nc) as tc:
        with (
            tc.tile_pool(name="const", bufs=1) as cp,
            tc.tile_pool(name="sc", bufs=2) as scp,
            tc.tile_pool(name="bis", bufs=1) as bp,
            tc.tile_pool(name="ffn", bufs=2) as fp,
            tc.tile_pool(name="pk", bufs=2 if cfg.acc_bf16 else 1) as pkp,
            tc.tile_pool(name="norm", bufs=2) as np_,
            tc.tile_pool(name="pmm", bufs=4, space="PSUM") as pmm,
            tc.tile_pool(name="pps", bufs=2, space="PSUM") as pps,
        ):
            # ---------- phase 0: zero dense accumulators (scalar queue) ---
            ZR = 8 if cfg.acc_bf16 else 4
            zt = cp.tile([128, ZR * ROW], ACC, tag="zt")
            nc.vector.memset(zt[:], 0.0)
            for b in range(B):
                dz = dense_b[b].ap().rearrange(
                    "(j p zr) r -> j p (zr r)", p=128, zr=ZR)
                for j in range(T // (128 * ZR)):
                    nc.scalar.dma_start(dz[j], zt[:])

            # ---------- load constants / weights ----------
            wg_sb = cp.tile([128, DC, E], F32, tag="wg_sb")
            nc.sync.dma_start(wg_sb[:], wg_d.ap().rearrange("(c p) e -> p c e", p=128))
            w1_sb = cp.tile([128, DC, H], BF16, tag="w1_sb")
            nc.sync.dma_start(w1_sb[:], w1_d.ap().rearrange("(c p) h -> p c h", p=128))
            w2_sb = cp.tile([128, HC, D], BF16, tag="w2_sb")
            nc.sync.dma_start(w2_sb[:], w2_d.ap().rearrange("(c p) d -> p c d", p=128))
            blks = cp.tile([128, 128], F32, tag="blks")
            nc.sync.dma_start(blks[:], blk_d.ap())
            e1bs = cp.tile([128, B], F32, tag="e1bs")
            nc.sync.dma_start(e1bs[:], e1b_d.ap())
            iotap1 = cp.tile([16, B * TB16], F32, tag="iotap1")
            nc.sync.dma_start(iotap1[:], iotap1_d.ap())
            id4s = cp.tile([B, B], I32, tag="id4s")
            nc.sync.dma_start(id4s[:], id4_d.ap())
            o416s = cp.tile([B, 16], F32, tag="o416s")
            nc.sync.dma_start(o416s[:], o416_d.ap())
            id16s = cp.tile([16, 16], F32, tag="id16s")
            nc.sync.dma_start(id16s[:], id16_d.ap())
            idbfs = cp.tile([128, 128], BF16, tag="idbfs")
            nc.sync.dma_start(idbfs[:], idbf_d.ap())
            o16s = cp.tile([16, 1], F32, tag="o16s")
            nc.sync.dma_start(o16s[:], o16_d.ap())
            mks = cp.tile([CF, 128], F32, tag="mks")
            nc.sync.dma_start(mks[:], mk_d.ap())
            rsels = cp.tile([CF, CS], F32, tag="rsels")
            nc.sync.dma_start(rsels[:], rsel_d.ap())

            # ---------- phase 1: partial scores for my token shard -------
            for nt in range(TSH // 512):
                xt_t = scp.tile([128, DC, 512], F32, tag="xt")
                nc.sync.dma_start(
                    xt_t[:],
                    xt_sh.ap().rearrange("(c p) t -> p c t", p=128)[
                        :, :, nt * 512:(nt + 1) * 512],
                )
                ps_sc = pps.tile([E, 512], F32, tag="sp")
                for dc in range(DC):
                    nc.tensor.matmul(ps_sc[:], lhsT=wg_sb[:, dc, :],
                                     rhs=xt_t[:, dc, :],
                                     start=(dc == 0), stop=(dc == DC - 1))
                sc_sb = scp.tile([E, 512], F32, tag="scsb")
                nc.vector.tensor_copy(sc_sb[:], ps_sc[:])
                nc.sync.dma_start(a2a_in[:, nt * 512:(nt + 1) * 512], sc_sb[:])

            # ---------- phase 2: AllToAll ----------
            nc.gpsimd.collective_compute(
                "AllToAll", ALU.bypass, replica_groups=[list(range(NCORES))],
                ins=[a2a_in.ap()], outs=[a2a_out.ap()],
            )

            PPR = 128 // E
            w128 = cp.tile([128, TPP], F32, tag="w128")
            for r in range(E):
                nc.sync.dma_start(
                    w128[r * PPR:(r + 1) * PPR, :],
                    a2a_out.ap()[r].rearrange("(l f) -> l f", l=PPR))
            w16 = cp.tile([16, B * TB16], F32, tag="w16")
            for r in range(E):
                b, q = divmod(r, RPB)
                nc.sync.dma_start(
                    w16[:, b * TB16 + q * QL: b * TB16 + (q + 1) * QL],
                    a2a_out.ap()[r].rearrange("(s j) -> s j", s=16))

            # ---------- phase 3: softmax pieces ----------
            exp16 = cp.tile([16, B * TB16], F32, tag="exp16")
            parts16 = bp.tile([16, B], F32, tag="parts16")
            for b in range(B):
                sl = slice(b * TB16, (b + 1) * TB16)
                nc.scalar.activation(exp16[:, sl], w16[:, sl], AF.Exp,
                                     accum_out=parts16[:, b:b + 1])
            ps4 = pps.tile([B, 1], F32, tag="sp")
            nc.tensor.matmul(ps4[:], lhsT=parts16[:], rhs=o16s[:],
                             start=True, stop=True)
            recip4 = bp.tile([B, 1], F32, tag="recip4")
            nc.vector.reciprocal(recip4[:], ps4[:])
            diagr = bp.tile([B, B], F32, tag="diagr")
            nc.vector.memset(diagr[:], 0.0)
            nc.vector.copy_predicated(diagr[:], id4s[:],
                                      recip4[:, 0:1].to_broadcast([B, B]))
            psr16 = pps.tile([16, B], F32, tag="sp")
            nc.tensor.matmul(psr16[:], lhsT=o416s[:], rhs=diagr[:],
                             start=True, stop=True)
            recip16 = cp.tile([16, B], F32, tag="recip16")
            nc.vector.tensor_copy(recip16[:], psr16[:])

            # ---------- phase 4: threshold bisection ----------
            lo128 = bp.tile([128, 1], F32, tag="lo128")
            hi128 = bp.tile([128, 1], F32, tag="hi128")
            nc.vector.memset(lo128[:], -cfg.span)
            nc.vector.memset(hi128[:], cfg.span)
            mid128 = bp.tile([128, 1], F32, tag="mid128")
            sel128 = bp.tile([128, 1], I32, tag="sel128")
            seli128 = bp.tile([128, 1], I32, tag="seli128")
            cnt128 = bp.tile([128, 1], F32, tag="cnt128")
            msk = bp.tile([128, TPP], F32, tag="msk")
            for _ in range(cfg.nrounds if cfg.stage >= 1 else 0):
                nc.vector.tensor_add(mid128[:], lo128[:], hi128[:])
                nc.vector.tensor_scalar_mul(mid128[:], mid128[:], 0.5)
                nc.vector.tensor_scalar(msk[:], w128[:], mid128[:, 0:1], None,
                                        op0=ALU.is_ge, op1=ALU.add,
                                        accum_out=cnt128[:, 0:1])
                ptot = pps.tile([128, 1], F32, tag="sp")
                nc.tensor.matmul(ptot[:], lhsT=blks[:], rhs=cnt128[:],
                                 start=True, stop=True)
                nc.vector.tensor_scalar(sel128[:], ptot[:], float(C) - 0.5,
                                        None, op0=ALU.is_ge)
                nc.vector.tensor_scalar(seli128[:], ptot[:], float(C) - 0.5,
                                        None, op0=ALU.is_lt)
                nc.vector.copy_predicated(lo128[:], sel128[:], mid128[:])
                nc.vector.copy_predicated(hi128[:], seli128[:], mid128[:])

            # tau4 / tau16
            ptau = pps.tile([B, 1], F32, tag="sp")
            nc.tensor.matmul(ptau[:], lhsT=e1bs[:], rhs=lo128[:],
                             start=True, stop=True)
            tau4 = bp.tile([B, 1], F32, tag="tau4")
            nc.vector.tensor_copy(tau4[:], ptau[:])
            diagt = bp.tile([B, B], F32, tag="diagt")
            nc.vector.memset(diagt[:], 0.0)
            nc.vector.copy_predicated(diagt[:], id4s[:],
                                      tau4[:, 0:1].to_broadcast([B, B]))
            pst16 = pps.tile([16, B], F32, tag="sp")
            nc.tensor.matmul(pst16[:], lhsT=o416s[:], rhs=diagt[:],
                             start=True, stop=True)
            tau16 = cp.tile([16, B], F32, tag="tau16")
            nc.vector.tensor_copy(tau16[:], pst16[:])

            # ---------- phase 5: compaction + 16->128 relayout (v1) ------
            idxg32s, idxl32s, val128s, selTs = [], [], [], []
            for b in range(B if cfg.stage >= 2 else 0):
                sl = slice(b * TB16, (b + 1) * TB16)
                mask16 = bp.tile([16, TB16], F32, tag="mask16")
                nc.vector.tensor_scalar(mask16[:], w16[:, sl], tau16[:, b:b + 1],
                                        None, op0=ALU.is_ge)
                candi = bp.tile([16, TB16], F32, tag="candi")
                nc.vector.tensor_tensor(candi[:], mask16[:], iotap1[:, sl],
                                        op=ALU.mult)
                nc.vector.tensor_scalar_add(candi[:], candi[:], -1.0)
                candv = bp.tile([16, TB16], F32, tag="candv")
                nc.vector.tensor_tensor(candv[:], mask16[:], exp16[:, sl],
                                        op=ALU.mult)
                nc.vector.tensor_scalar_add(mask16[:], mask16[:], -1.0)
                nc.vector.tensor_tensor(candv[:], candv[:], mask16[:],
                                        op=ALU.add)

                ci = bp.tile([16, CF + 16], F32, tag=f"ci{b}")
                nfi = bp.tile([1, 1], U32, tag=f"nfi{b}")
                nc.gpsimd.sparse_gather(ci[:], candi[:], num_found=nfi[:])
                cv = bp.tile([16, CF + 16], F32, tag=f"cv{b}")
                nfv = bp.tile([1, 1], U32, tag=f"nfv{b}")
                nc.gpsimd.sparse_gather(cv[:], candv[:], num_found=nfv[:])
                nc.sync.dma_start(nf_out.ap()[b:b + 1, 0:1], nfi[:, :])
                nc.sync.dma_start(nf_out.ap()[b:b + 1, 1:2], nfv[:, :])

                nc.vector.tensor_scalar(cv[:, :CF], cv[:, :CF],
                                        recip16[:, b:b + 1], None, op0=ALU.mult)
                nc.vector.tensor_scalar_add(ci[:, :CF], ci[:, :CF],
                                            float(b * T))

                pti = pps.tile([CF, 16], F32, tag="sp")
                nc.tensor.transpose(pti[:], ci[:, :CF], id16s[:])
                cit = bp.tile([CF, 16], F32, tag="cit")
                nc.vector.tensor_copy(cit[:], pti[:])
                ptv = pps.tile([CF, 16], F32, tag="sp")
                nc.tensor.transpose(ptv[:], cv[:, :CF], id16s[:])
                cvt = bp.tile([CF, 16], F32, tag="cvt")
                nc.vector.tensor_copy(cvt[:], ptv[:])

                cmi = bp.tile([CF, 128], F32, tag="cmi")
                nc.vector.tensor_tensor(
                    cmi[:].rearrange("f (g s) -> f g s", g=8),
                    cit[:, None, :].to_broadcast([CF, 8, 16]),
                    mks[:].rearrange("f (g s) -> f g s", g=8),
                    op=ALU.mult)
                cmv = bp.tile([CF, 128], F32, tag="cmv")
                nc.vector.tensor_tensor(
                    cmv[:].rearrange("f (g s) -> f g s", g=8),
                    cvt[:, None, :].to_broadcast([CF, 8, 16]),
                    mks[:].rearrange("f (g s) -> f g s", g=8),
                    op=ALU.mult)

                pri = pps.tile([128, CS], F32, tag="sp")
                nc.tensor.matmul(pri[:], lhsT=cmi[:], rhs=rsels[:],
                                 start=True, stop=True)
                idxg32 = cp.tile([128, CS], I32, name=f"idxg_{b}",
                                 tag=f"idxg_{b}")
                nc.vector.tensor_scalar(idxg32[:], pri[:], 0.0, None,
                                        op0=ALU.max)
                idxl32 = cp.tile([128, CS], I32, name=f"idxl_{b}",
                                 tag=f"idxl_{b}")
                nc.vector.tensor_scalar(idxl32[:], pri[:], float(-b * T),
                                        None, op0=ALU.add)
                prv = pps.tile([128, CS], F32, tag="sp")
                nc.tensor.matmul(prv[:], lhsT=cmv[:], rhs=rsels[:],
                                 start=True, stop=True)
                val128 = cp.tile([128, CS], F32, name=f"val_{b}",
                                 tag=f"val_{b}")
                nc.vector.tensor_copy(val128[:], prv[:])
                idxg32s.append(idxg32)
                idxl32s.append(idxl32)
                val128s.append(val128)

                # gather + transpose now, BEFORE any ReduceScatter is in
                # flight: PE transposes are serialized against collectives
                # by the tile framework, which stalled the FFN ~50us/batch.
                if cfg.stage >= 3:
                    selTM = fp.tile([128, CS, D], BF16, tag="selTM")
                    for cs in range(CS):
                        nc.gpsimd.indirect_dma_start(
                            out=selTM[:, cs, :],
                            out_offset=None,
                            in_=x_bf.ap(),
                            in_offset=IndirectOffsetOnAxis(
                                ap=idxg32[:, cs:cs + 1], axis=0))
                    selT = cp.tile([128, DC, C], BF16, name=f"selT_{b}",
                                   tag=f"selT_{b}")
                    for cs in range(CS):
                        for dc in range(DC):
                            ptp = pps.tile([128, 128], BF16, tag="tp")
                            nc.tensor.transpose(
                                ptp[:], selTM[:, cs, dc * 128:(dc + 1) * 128],
                                idbfs[:])
                            nc.vector.tensor_copy(
                                selT[:, dc, cs * 128:(cs + 1) * 128], ptp[:])
                    selTs.append(selT)

            # ---------- phase 6: per-batch FFN + scatter + inline RS -----
            for b in range(B if cfg.stage >= 3 else 0):
                selT = selTs[b]
                pk = pkp.tile([128, CS, ROW], ACC, tag="pk")
                nc.vector.memset(pk[:], 0.0)
                for ct in range(C // NT):
                    csl = slice(ct * NT, (ct + 1) * NT)
                    hT = fp.tile([128, HC, NT], BF16, tag="hT")
                    for ht in range(HC):
                        psh = pmm.tile([128, NT], F32, tag="mm")
                        for dc in range(DC):
                            nc.tensor.matmul(
                                psh[:],
                                lhsT=w1_sb[:, dc, ht * 128:(ht + 1) * 128],
                                rhs=selT[:, dc, csl],
                                start=(dc == 0), stop=(dc == DC - 1))
                        nc.scalar.activation(hT[:, ht, :], psh[:],
                                             getattr(AF, cfg.act))
                    for cl in range(NT // 128):
                        cs = ct * (NT // 128) + cl
                        pso = pmm.tile([128, D], F32, tag="mm")
                        for hc in range(HC):
                            nc.tensor.matmul(
                                pso[:],
                                lhsT=hT[:, hc, cl * 128:(cl + 1) * 128],
                                rhs=w2_sb[:, hc, :],
                                start=(hc == 0), stop=(hc == HC - 1))
                        nc.vector.tensor_scalar(
                            pk[:, cs, :D], pso[:],
                            val128s[b][:, cs:cs + 1], None, op0=ALU.mult)
                        nc.vector.tensor_copy(pk[:, cs, D:D + 1],
                                              val128s[b][:, cs:cs + 1])
                for cs in range(CS):
                    nc.gpsimd.indirect_dma_start(
                        out=dense_b[b].ap(),
                        out_offset=IndirectOffsetOnAxis(
                            ap=idxl32s[b][:, cs:cs + 1], axis=0),
                        in_=pk[:, cs, :],
                        in_offset=None,
                        bounds_check=T - 1,
                        oob_is_err=False)
                if cfg.rs_inline and cfg.stage >= 4:
                    nc.gpsimd.collective_compute(
                        "ReduceScatter", ALU.add,
                        replica_groups=[list(range(NCORES))],
                        ins=[dense_b[b].ap()], outs=[rs_b[b].ap()],
                    )
            if not cfg.rs_inline and cfg.stage >= 4:
                for b in range(B):
                    nc.gpsimd.collective_compute(
                        "ReduceScatter", ALU.add,
                        replica_groups=[list(range(NCORES))],
                        ins=[dense_b[b].ap()], outs=[rs_b[b].ap()],
                    )

            # ---------- phase 7: normalize my shard of each batch --------
            for b in range(B if cfg.stage >= 4 else 0):
                for j in range(TQ // 128):
                    rsl = slice(j * 128, (j + 1) * 128)
                    ld = np_.tile([128, D + 1], ACC, tag="ld")
                    nc.sync.dma_start(ld[:], rs_b[b].ap()[rsl, :D + 1])
                    dn = np_.tile([128, 1], F32, tag="dn")
                    nc.vector.tensor_scalar(dn[:], ld[:, D:D + 1], 1e-8, None,
                                            op0=ALU.max)
                    rc = np_.tile([128, 1], F32, tag="rc")
                    nc.vector.reciprocal(rc[:], dn[:])
                    ot = np_.tile([128, D], F32, tag="ot")
                    nc.vector.tensor_scalar(ot[:], ld[:, :D], rc[:, 0:1], None,
                                            op0=ALU.mult)
                    nc.sync.dma_start(
                        out_sh.ap()[b * TQ + j * 128: b * TQ + (j + 1) * 128, :],
                        ot[:])

    nc.compile()
    return nc


# ---------------------------------------------------------------------------
# host side
# ---------------------------------------------------------------------------

def host_consts(cfg: Cfg = FULL):
    B, T = cfg.B, cfg.T
    TB16, RPB, QL, CF, CS = cfg.TB16, cfg.RPB, cfg.QL, cfg.CF, cfg.CS
    p = np.arange(128)
    blk = (p[:, None] // 32 == p[None, :] // 32).astype(np.float32)
    e1b = (p[:, None] // 32 == np.arange(B)[None, :]).astype(np.float32) / 32.0
    iotap1 = np.zeros((16, B * TB16), np.float32)
    for s in range(16):
        for q in range(RPB):
            j = np.arange(QL)
            t = q * (T // RPB) + s * QL + j
            for b in range(B):
                iotap1[s, b * TB16 + q * QL + j] = t + 1
    id4 = np.eye(B, dtype=np.int32)
    o416 = np.ones((B, 16), np.float32)
    id16 = np.eye(16, dtype=np.float32)
    idbf = np.eye(128).astype(ml_dtypes.bfloat16)
    o16 = np.ones((16, 1), np.float32)
    f = np.arange(CF)
    g = np.arange(8)
    mk = np.zeros((CF, 128), np.float32)
    mk.reshape(CF, 8, 16)[:, :, :] = (f[:, None] % 8 == g[None, :]).astype(
        np.float32)[:, :, None]
    rsel = (f[:, None] // 8 == np.arange(CS)[None, :]).astype(np.float32)
    return dict(blk128=blk, e1b=e1b, iotap1=iotap1, id4=id4, o416=o416,
                id16=id16, idbf=idbf, o16=o16, mk=mk, rsel=rsel)


def make_in_maps(inputs, cfg: Cfg = FULL):
    x = np.asarray(inputs["x"], np.float32).reshape(cfg.BT, cfg.D)
    Wg = np.ascontiguousarray(np.asarray(inputs["Wg"], np.float32))
    W1 = np.asarray(inputs["W1"], np.float32)
    W2 = np.asarray(inputs["W2"], np.float32)
    consts = host_consts(cfg)
    x_bf = x.astype(ml_dtypes.bfloat16)
    in_maps = []
    for i in range(NCORES):
        m = dict(consts)
        m["x_bf"] = x_bf
        m["xt_sh"] = np.ascontiguousarray(x[i * cfg.TSH:(i + 1) * cfg.TSH].T)
        m["wg"] = Wg
        m["w1"] = np.ascontiguousarray(W1[i].astype(ml_dtypes.bfloat16))
        m["w2"] = np.ascontiguousarray(W2[i].astype(ml_dtypes.bfloat16))
        in_maps.append(m)
    return in_maps


def assemble_out(results, cfg: Cfg = FULL):
    nf = np.stack([np.asarray(results[i]["nf_out"]) for i in range(NCORES)])
    if not (nf == cfg.C).all():
        print(f"WARNING: sparse_gather num_found != {cfg.C}: {nf.tolist()}",
              file=sys.stderr)
    TQ = 1024
    out = np.empty((cfg.B, cfg.T, cfg.D), np.float32)
    for i in range(NCORES):
        r = np.asarray(results[i]["out_sh"]).reshape(cfg.B, TQ, cfg.D)
        for b in range(cfg.B):
            out[b, i * TQ:(i + 1) * TQ] = r[b]
    return out


_NC_CACHE = {}


def get_nc():
    if "nc" not in _NC_CACHE:
        _NC_CACHE["nc"] = build_nc(_NC_CACHE.get("cfg", FULL))
    return _NC_CACHE["nc"]


def kernel(**inputs):
    nc = get_nc()
    in_maps = make_in_maps(inputs, _NC_CACHE.get("cfg", FULL))
    res = run_bass_kernel_spmd(nc, in_maps, core_ids=list(range(NCORES)),
                               **_NC_CACHE.get("run_kwargs", {}))
    _NC_CACHE["last_run"] = res
    return assemble_out(res.results, FULL)


# revision 72
# speedup vs baseline: 1.0296x; 1.0027x over previous
"""Expert-choice MoE router kernel for Trainium2 (8 NeuronCores), v3.

Problem (B=4, T=8192, D=512, E=8, H=2048, C=1024):
  scores = x @ Wg; w = softmax over T per (b,e); top-C tokens per (b,e);
  y = gelu(x[sel] @ W1) @ W2 * w[sel]; out = scatter_add(y)/max(sum w, 1e-8)

Sharding: expert-parallel, one expert per core.

v3 = v1's proven selection/compaction machinery + the v2 wins that were
individually validated on HW:
  - dense accumulator: bf16, split per batch (4 tensors), zeroed on the
    scalar-engine DMA queue (v1 lost 200 us blocking the sync queue).
  - ReduceScatter: bf16, per batch, issued inside the FFN loop so RS(b)
    overlaps FFN(b+1).
  - bisection: 28 rounds over [-8, 8] (max|score| ~4.9, top-C gap 3.7e-6
    >> 16/2^28), one PE trip per round via block-diag segment-sum.
  - compaction: v1 verbatim (two sparse_gathers per batch on (16, 512)
    slices + selection-matmul relayout) -- the v2 paired variant crashes
    the exec unit on HW.
"""

import sys
from dataclasses import dataclass

sys.path.insert(0, "/opt/trn_rl_repo")

import numpy as np
import ml_dtypes

import concourse.bass as bass  # noqa: F401
import concourse.mybir as mybir
import concourse.tile as tile
from concourse import bacc
from concourse.bass import IndirectOffsetOnAxis
from concourse.bass_utils import run_bass_kernel_spmd

F32 = mybir.dt.float32
BF16 = mybir.dt.bfloat16
I32 = mybir.dt.int32
U32 = mybir.dt.uint32
AF = mybir.ActivationFunctionType
ALU = mybir.AluOpType

NCORES = 8


@dataclass(frozen=True)
class Cfg:
    B: int = 4
    T: int = 8192
    D: int = 512
    E: int = 8
    H: int = 2048
    C: int = 1024
    nrounds: int = 28
    span: float = 8.0
    act: str = "Gelu"
    acc_bf16: bool = True
    rs_inline: bool = True
    stage: int = 4   # 1=+bisect, 2=+selection, 3=+FFN, 4=full

    @property
    def BT(self):
        return self.B * self.T

    @property
    def TSH(self):
        return self.BT // NCORES

    @property
    def ROW(self):
        return self.D + 8

    @property
    def DC(self):
        return self.D // 128

    @property
    def HC(self):
        return self.H // 128

    @property
    def TPP(self):
        return self.T * self.B // 128

    @property
    def RPB(self):
        return NCORES // self.B

    @property
    def QL(self):
        return self.T // self.RPB // 16

    @property
    def TB16(self):
        return self.T // 16

    @property
    def CF(self):
        return self.C // 16

    @property
    def CS(self):
        return self.C // 128


FULL = Cfg()


def build_nc(cfg: Cfg = FULL):
    B, T, D, E, H, C = cfg.B, cfg.T, cfg.D, cfg.E, cfg.H, cfg.C
    BT, TSH, ROW, DC, HC = cfg.BT, cfg.TSH, cfg.ROW, cfg.DC, cfg.HC
    TPP, RPB, QL, TB16 = cfg.TPP, cfg.RPB, cfg.QL, cfg.TB16
    CF, CS = cfg.CF, cfg.CS
    NT = 512
    TQ = 1024               # rows per core of a per-batch ReduceScatter
    ACC = BF16 if cfg.acc_bf16 else F32

    nc = bacc.Bacc("TRN2", target_bir_lowering=False, debug=False,
                   num_devices=NCORES)

    # ---- I/O ----
    x_bf = nc.dram_tensor("x_bf", [BT, D], BF16, kind="ExternalInput")
    xt_sh = nc.dram_tensor("xt_sh", [D, TSH], F32, kind="ExternalInput")
    wg_d = nc.dram_tensor("wg", [D, E], F32, kind="ExternalInput")
    w1_d = nc.dram_tensor("w1", [D, H], BF16, kind="ExternalInput")
    w2_d = nc.dram_tensor("w2", [H, D], BF16, kind="ExternalInput")
    blk_d = nc.dram_tensor("blk128", [128, 128], F32, kind="ExternalInput")
    e1b_d = nc.dram_tensor("e1b", [128, B], F32, kind="ExternalInput")
    iotap1_d = nc.dram_tensor("iotap1", [16, B * TB16], F32,
                              kind="ExternalInput")
    id4_d = nc.dram_tensor("id4", [B, B], I32, kind="ExternalInput")
    o416_d = nc.dram_tensor("o416", [B, 16], F32, kind="ExternalInput")
    id16_d = nc.dram_tensor("id16", [16, 16], F32, kind="ExternalInput")
    idbf_d = nc.dram_tensor("idbf", [128, 128], BF16, kind="ExternalInput")
    o16_d = nc.dram_tensor("o16", [16, 1], F32, kind="ExternalInput")
    mk_d = nc.dram_tensor("mk", [CF, 128], F32, kind="ExternalInput")
    rsel_d = nc.dram_tensor("rsel", [CF, CS], F32, kind="ExternalInput")

    out_sh = nc.dram_tensor("out_sh", [TSH, D], F32, kind="ExternalOutput")
    nf_out = nc.dram_tensor("nf_out", [B, 2], U32, kind="ExternalOutput")

    # ---- internal DRAM ----
    a2a_in = nc.dram_tensor("a2a_in", [E, TSH], F32)
    a2a_out = nc.dram_tensor("a2a_out", [E, TSH], F32)
    dense_b = [nc.dram_tensor(f"dense{b}", [T, ROW], ACC) for b in range(B)]
    rs_b = [nc.dram_tensor(f"rs{b}", [TQ, ROW], ACC) for b in range(B)]

    with tile.TileContext(nc) as tc:
        with (
            tc.tile_pool(name="const", bufs=1) as cp,
            tc.tile_pool(name="sc", bufs=2) as scp,
            tc.tile_pool(name="bis", bufs=1) as bp,
            tc.tile_pool(name="ffn", bufs=2) as fp,
            tc.tile_pool(name="pk", bufs=2 if cfg.acc_bf16 else 1) as pkp,
            tc.tile_pool(name="norm", bufs=2) as np_,
            tc.tile_pool(name="pmm", bufs=4, space="PSUM") as pmm,
            tc.tile_pool(name="pps", bufs=2, space="PSUM") as pps,
        ):
            # ---------- phase 0: zero dense accumulators (scalar queue) ---
            ZR = 8 if cfg.acc_bf16 else 4
            zt = cp.tile([128, ZR * ROW], ACC, tag="zt")
            nc.vector.memset(zt[:], 0.0)
            for b in range(B):
                dz = dense_b[b].ap().rearrange(
                    "(j p zr) r -> j p (zr r)", p=128, zr=ZR)
                for j in range(T // (128 * ZR)):
                    nc.scalar.dma_start(dz[j], zt[:])

            # ---------- load constants / weights ----------
            wg_sb = cp.tile([128, DC, E], F32, tag="wg_sb")
            nc.sync.dma_start(wg_sb[:], wg_d.ap().rearrange("(c p) e -> p c e", p=128))
            w1_sb = cp.tile([128, DC, H], BF16, tag="w1_sb")
            nc.sync.dma_start(w1_sb[:], w1_d.ap().rearrange("(c p) h -> p c h", p=128))
            w2_sb = cp.tile([128, HC, D], BF16, tag="w2_sb")
            nc.sync.dma_start(w2_sb[:], w2_d.ap().rearrange("(c p) d -> p c d", p=128))
            blks = cp.tile([128, 128], F32, tag="blks")
            nc.sync.dma_start(blks[:], blk_d.ap())
            e1bs = cp.tile([128, B], F32, tag="e1bs")
            nc.sync.dma_start(e1bs[:], e1b_d.ap())
            iotap1 = cp.tile([16, B * TB16], F32, tag="iotap1")
            nc.sync.dma_start(iotap1[:], iotap1_d.ap())
            id4s = cp.tile([B, B], I32, tag="id4s")
            nc.sync.dma_start(id4s[:], id4_d.ap())
            o416s = cp.tile([B, 16], F32, tag="o416s")
            nc.sync.dma_start(o416s[:], o416_d.ap())
            id16s = cp.tile([16, 16], F32, tag="id16s")
            nc.sync.dma_start(id16s[:], id16_d.ap())
            idbfs = cp.tile([128, 128], BF16, tag="idbfs")
            nc.sync.dma_start(idbfs[:], idbf_d.ap())
            o16s = cp.tile([16, 1], F32, tag="o16s")
            nc.sync.dma_start(o16s[:], o16_d.ap())
            mks = cp.tile([CF, 128], F32, tag="mks")
            nc.sync.dma_start(mks[:], mk_d.ap())
            rsels = cp.tile([CF, CS], F32, tag="rsels")
            nc.sync.dma_start(rsels[:], rsel_d.ap())

            # ---------- phase 1: partial scores for my token shard -------
            for nt in range(TSH // 512):
                xt_t = scp.tile([128, DC, 512], F32, tag="xt")
                nc.sync.dma_start(
                    xt_t[:],
                    xt_sh.ap().rearrange("(c p) t -> p c t", p=128)[
                        :, :, nt * 512:(nt + 1) * 512],
                )
                ps_sc = pps.tile([E, 512], F32, tag="sp")
                for dc in range(DC):
                    nc.tensor.matmul(ps_sc[:], lhsT=wg_sb[:, dc, :],
                                     rhs=xt_t[:, dc, :],
                                     start=(dc == 0), stop=(dc == DC - 1))
                sc_sb = scp.tile([E, 512], F32, tag="scsb")
                nc.vector.tensor_copy(sc_sb[:], ps_sc[:])
                nc.sync.dma_start(a2a_in[:, nt * 512:(nt + 1) * 512], sc_sb[:])

            # ---------- phase 2: AllToAll ----------
            nc.gpsimd.collective_compute(
                "AllToAll", ALU.bypass, replica_groups=[list(range(NCORES))],
                ins=[a2a_in.ap()], outs=[a2a_out.ap()],
            )

            PPR = 128 // E
            w128 = cp.tile([128, TPP], F32, tag="w128")
            for r in range(E):
                nc.sync.dma_start(
                    w128[r * PPR:(r + 1) * PPR, :],
                    a2a_out.ap()[r].rearrange("(l f) -> l f", l=PPR))
            w16 = cp.tile([16, B * TB16], F32, tag="w16")
            for r in range(E):
                b, q = divmod(r, RPB)
                nc.sync.dma_start(
                    w16[:, b * TB16 + q * QL: b * TB16 + (q + 1) * QL],
                    a2a_out.ap()[r].rearrange("(s j) -> s j", s=16))

            # ---------- phase 3: softmax pieces ----------
            exp16 = cp.tile([16, B * TB16], F32, tag="exp16")
            parts16 = bp.tile([16, B], F32, tag="parts16")
            for b in range(B):
                sl = slice(b * TB16, (b + 1) * TB16)
                nc.scalar.activation(exp16[:, sl], w16[:, sl], AF.Exp,
                                     accum_out=parts16[:, b:b + 1])
            ps4 = pps.tile([B, 1], F32, tag="sp")
            nc.tensor.matmul(ps4[:], lhsT=parts16[:], rhs=o16s[:],
                             start=True, stop=True)
            recip4 = bp.tile([B, 1], F32, tag="recip4")
            nc.vector.reciprocal(recip4[:], ps4[:])
            diagr = bp.tile([B, B], F32, tag="diagr")
            nc.vector.memset(diagr[:], 0.0)
            nc.vector.copy_predicated(diagr[:], id4s[:],
                                      recip4[:, 0:1].to_broadcast([B, B]))
            psr16 = pps.tile([16, B], F32, tag="sp")
            nc.tensor.matmul(psr16[:], lhsT=o416s[:], rhs=diagr[:],
                             start=True, stop=True)
            recip16 = cp.tile([16, B], F32, tag="recip16")
            nc.vector.tensor_copy(recip16[:], psr16[:])

            # ---------- phase 4: threshold bisection ----------
            lo128 = bp.tile([128, 1], F32, tag="lo128")
            hi128 = bp.tile([128, 1], F32, tag="hi128")
            nc.vector.memset(lo128[:], -cfg.span)
            nc.vector.memset(hi128[:], cfg.span)
            mid128 = bp.tile([128, 1], F32, tag="mid128")
            sel128 = bp.tile([128, 1], I32, tag="sel128")
            seli128 = bp.tile([128, 1], I32, tag="seli128")
            cnt128 = bp.tile([128, 1], F32, tag="cnt128")
            msk = bp.tile([128, TPP], F32, tag="msk")
            for _ in range(cfg.nrounds if cfg.stage >= 1 else 0):
                nc.vector.tensor_add(mid128[:], lo128[:], hi128[:])
                nc.vector.tensor_scalar_mul(mid128[:], mid128[:], 0.5)
                nc.vector.tensor_scalar(msk[:], w128[:], mid128[:, 0:1], None,
                                        op0=ALU.is_ge, op1=ALU.add,
                                        accum_out=cnt128[:, 0:1])
                ptot = pps.tile([128, 1], F32, tag="sp")
                nc.tensor.matmul(ptot[:], lhsT=blks[:], rhs=cnt128[:],
                                 start=True, stop=True)
                nc.vector.tensor_scalar(sel128[:], ptot[:], float(C) - 0.5,
                                        None, op0=ALU.is_ge)
                nc.vector.tensor_scalar(seli128[:], ptot[:], float(C) - 0.5,
                                        None, op0=ALU.is_lt)
                nc.vector.copy_predicated(lo128[:], sel128[:], mid128[:])
                nc.vector.copy_predicated(hi128[:], seli128[:], mid128[:])

            # tau4 / tau16
            ptau = pps.tile([B, 1], F32, tag="sp")
            nc.tensor.matmul(ptau[:], lhsT=e1bs[:], rhs=lo128[:],
                             start=True, stop=True)
            tau4 = bp.tile([B, 1], F32, tag="tau4")
            nc.vector.tensor_copy(tau4[:], ptau[:])
            diagt = bp.tile([B, B], F32, tag="diagt")
            nc.vector.memset(diagt[:], 0.0)
            nc.vector.copy_predicated(diagt[:], id4s[:],
                                      tau4[:, 0:1].to_broadcast([B, B]))
            pst16 = pps.tile([16, B], F32, tag="sp")
            nc.tensor.matmul(pst16[:], lhsT=o416s[:], rhs=diagt[:],
                             start=True, stop=True)
            tau16 = cp.tile([16, B], F32, tag="tau16")
            nc.vector.tensor_copy(tau16[:], pst16[:])

            # ---------- phase 5: compaction + 16->128 relayout (v1) ------
            idxg32s, idxl32s, val128s, selTs = [], [], [], []
            for b in range(B if cfg.stage >= 2 else 0):
                sl = slice(b * TB16, (b + 1) * TB16)
                mask16 = bp.tile([16, TB16], F32, tag="mask16")
                nc.vector.tensor_scalar(mask16[:], w16[:, sl], tau16[:, b:b + 1],
                                        None, op0=ALU.is_ge)
                candi = bp.tile([16, TB16], F32, tag="candi")
                nc.vector.tensor_tensor(candi[:], mask16[:], iotap1[:, sl],
                                        op=ALU.mult)
                nc.vector.tensor_scalar_add(candi[:], candi[:], -1.0)
                candv = bp.tile([16, TB16], F32, tag="candv")
                nc.vector.tensor_tensor(candv[:], mask16[:], exp16[:, sl],
                                        op=ALU.mult)
                nc.vector.tensor_scalar_add(mask16[:], mask16[:], -1.0)
                nc.vector.tensor_tensor(candv[:], candv[:], mask16[:],
                                        op=ALU.add)

                ci = bp.tile([16, CF + 16], F32, tag=f"ci{b}")
                nfi = bp.tile([1, 1], U32, tag=f"nfi{b}")
                nc.gpsimd.sparse_gather(ci[:], candi[:], num_found=nfi[:])
                cv = bp.tile([16, CF + 16], F32, tag=f"cv{b}")
                nfv = bp.tile([1, 1], U32, tag=f"nfv{b}")
                nc.gpsimd.sparse_gather(cv[:], candv[:], num_found=nfv[:])
                nc.sync.dma_start(nf_out.ap()[b:b + 1, 0:1], nfi[:, :])
                nc.sync.dma_start(nf_out.ap()[b:b + 1, 1:2], nfv[:, :])

                nc.vector.tensor_scalar(cv[:, :CF], cv[:, :CF],
                                        recip16[:, b:b + 1], None, op0=ALU.mult)
                nc.vector.tensor_scalar_add(ci[:, :CF], ci[:, :CF],
                                            float(b * T))

                pti = pps.tile([CF, 16], F32, tag="sp")
                nc.tensor.transpose(pti[:], ci[:, :CF], id16s[:])
                cit = bp.tile([CF, 16], F32, tag="cit")
                nc.vector.tensor_copy(cit[:], pti[:])
                ptv = pps.tile([CF, 16], F32, tag="sp")
                nc.tensor.transpose(ptv[:], cv[:, :CF], id16s[:])
                cvt = bp.tile([CF, 16], F32, tag="cvt")
                nc.vector.tensor_copy(cvt[:], ptv[:])

                cmi = bp.tile([CF, 128], F32, tag="cmi")
                nc.vector.tensor_tensor(
                    cmi[:].rearrange("f (g s) -> f g s", g=8),
                    cit[:, None, :].to_broadcast([CF, 8, 16]),
                    mks[:].rearrange("f (g s) -> f g s", g=8),
                    op=ALU.mult)
                cmv = bp.tile([CF, 128], F32, tag="cmv")
                nc.vector.tensor_tensor(
                    cmv[:].rearrange("f (g s) -> f g s", g=8),
                    cvt[:, None, :].to_broadcast([CF, 8, 16]),
                    mks[:].rearrange("f (g s) -> f g s", g=8),
                    op=ALU.mult)

                pri = pps.tile([128, CS], F32, tag="sp")
                nc.tensor.matmul(pri[:], lhsT=cmi[:], rhs=rsels[:],
                                 start=True, stop=True)
                idxg32 = cp.tile([128, CS], I32, name=f"idxg_{b}",
                                 tag=f"idxg_{b}")
                nc.vector.tensor_scalar(idxg32[:], pri[:], 0.0, None,
                                        op0=ALU.max)
                idxl32 = cp.tile([128, CS], I32, name=f"idxl_{b}",
                                 tag=f"idxl_{b}")
                nc.vector.tensor_scalar(idxl32[:], pri[:], float(-b * T),
                                        None, op0=ALU.add)
                prv = pps.tile([128, CS], F32, tag="sp")
                nc.tensor.matmul(prv[:], lhsT=cmv[:], rhs=rsels[:],
                                 start=True, stop=True)
                val128 = cp.tile([128, CS], F32, name=f"val_{b}",
                                 tag=f"val_{b}")
                nc.vector.tensor_copy(val128[:], prv[:])
                idxg32s.append(idxg32)
                idxl32s.append(idxl32)
                val128s.append(val128)

                # gather + transpose now, BEFORE any ReduceScatter is in
                # flight: PE transposes are serialized against collectives
                # by the tile framework, which stalled the FFN ~50us/batch.
                if cfg.stage >= 3:
                    selTM = fp.tile([128, CS, D], BF16, tag="selTM")
                    for cs in range(CS):
                        nc.gpsimd.indirect_dma_start(
                            out=selTM[:, cs, :],
                            out_offset=None,
                            in_=x_bf.ap(),
                            in_offset=IndirectOffsetOnAxis(
                                ap=idxg32[:, cs:cs + 1], axis=0))
                    selT = cp.tile([128, DC, C], BF16, name=f"selT_{b}",
                                   tag=f"selT_{b}")
                    for cs in range(CS):
                        for dc in range(DC):
                            ptp = pps.tile([128, 128], BF16, tag="tp")
                            nc.tensor.transpose(
                                ptp[:], selTM[:, cs, dc * 128:(dc + 1) * 128],
                                idbfs[:])
                            nc.vector.tensor_copy(
                                selT[:, dc, cs * 128:(cs + 1) * 128], ptp[:])
                    selTs.append(selT)

            # ---------- phase 6: per-batch FFN + scatter + inline RS -----
            for b in range(B if cfg.stage >= 3 else 0):
                selT = selTs[b]
                pk = pkp.tile([128, CS, ROW], ACC, tag="pk")
                nc.vector.memset(pk[:], 0.0)
                for ct in range(C // NT):
                    csl = slice(ct * NT, (ct + 1) * NT)
                    hT = fp.tile([128, HC, NT], BF16, tag="hT")
                    for ht in range(HC):
                        psh = pmm.tile([128, NT], F32, tag="mm")
                        for dc in range(DC):
                            nc.tensor.matmul(
                                psh[:],
                                lhsT=w1_sb[:, dc, ht * 128:(ht + 1) * 128],
                                rhs=selT[:, dc, csl],
                                start=(dc == 0), stop=(dc == DC - 1))
                        nc.scalar.activation(hT[:, ht, :], psh[:],
                                             getattr(AF, cfg.act))
                    for cl in range(NT // 128):
                        cs = ct * (NT // 128) + cl
                        pso = pmm.tile([128, D], F32, tag="mm")
                        for hc in range(HC):
                            nc.tensor.matmul(
                                pso[:],
                                lhsT=hT[:, hc, cl * 128:(cl + 1) * 128],
                                rhs=w2_sb[:, hc, :],
                                start=(hc == 0), stop=(hc == HC - 1))
                        nc.vector.tensor_scalar(
                            pk[:, cs, :D], pso[:],
                            val128s[b][:, cs:cs + 1], None, op0=ALU.mult)
                        nc.vector.tensor_copy(pk[:, cs, D:D + 1],
                                              val128s[b][:, cs:cs + 1])
                for cs in range(CS):
                    nc.gpsimd.indirect_dma_start(
                        out=dense_b[b].ap(),
                        out_offset=IndirectOffsetOnAxis(
                            ap=idxl32s[b][:, cs:cs + 1], axis=0),
                        in_=pk[:, cs, :],
                        in_offset=None,
                        bounds_check=T - 1,
                        oob_is_err=False)
                if cfg.rs_inline and cfg.stage >= 4:
                    nc.gpsimd.collective_compute(
                        "ReduceScatter", ALU.add,
                        replica_groups=[list(range(NCORES))],
                        ins=[dense_b[b].ap()], outs=[rs_b[b].ap()],
                    )
            if not cfg.rs_inline and cfg.stage >= 4:
                for b in range(B):
                    nc.gpsimd.collective_compute(
                        "ReduceScatter", ALU.add,
                        replica_groups=[list(range(NCORES))],
                        ins=[dense_b[b].ap()], outs=[rs_b[b].ap()],
                    )

            # ---------- phase 7: normalize my shard of each batch --------
            # Scheduled at the very end (priority pushed +1e6): its DVE ops
            # depend on RS outputs, and if interleaved into the FFN-era DVE
            # stream they stall the in-order DVE queue, which stalls the PE
            # via PSUM WAR edges (~47us per batch).
            with tc.high_priority(offset=-1000000):
                for b in range(B if cfg.stage >= 4 else 0):
                    for j in range(TQ // 128):
                        rsl = slice(j * 128, (j + 1) * 128)
                        ld = np_.tile([128, D + 1], ACC, tag="ld")
                        nc.sync.dma_start(ld[:], rs_b[b].ap()[rsl, :D + 1])
                        dn = np_.tile([128, 1], F32, tag="dn")
                        nc.vector.tensor_scalar(dn[:], ld[:, D:D + 1], 1e-8,
                                                None, op0=ALU.max)
                        rc = np_.tile([128, 1], F32, tag="rc")
                        nc.vector.reciprocal(rc[:], dn[:])
                        ot = np_.tile([128, D], F32, tag="ot")
                        nc.vector.tensor_scalar(ot[:], ld[:, :D], rc[:, 0:1],
                                                None, op0=ALU.mult)
                        nc.sync.dma_start(
                            out_sh.ap()[b * TQ + j * 128:
                                        b * TQ + (j + 1) * 128, :],
                            ot[:])

    nc.compile()
    return nc


# ---------------------------------------------------------------------------
# host side
# ---------------------------------------------------------------------------

def host_consts(cfg: Cfg = FULL):
    B, T = cfg.B, cfg.T
    TB16, RPB, QL, CF, CS = cfg.TB16, cfg.RPB, cfg.QL, cfg.CF, cfg.CS
    p = np.arange(128)
    blk = (p[:, None] // 32 == p[None, :] // 32).astype(np.float32)
    e1b = (p[:, None] // 32 == np.arange(B)[None, :]).astype(np.float32) / 32.0
    iotap1 = np.zeros((16, B * TB16), np.float32)
    for s in range(16):
        for q in range(RPB):
            j = np.arange(QL)
            t = q * (T // RPB) + s * QL + j
            for b in range(B):
                iotap1[s, b * TB16 + q * QL + j] = t + 1
    id4 = np.eye(B, dtype=np.int32)
    o416 = np.ones((B, 16), np.float32)
    id16 = np.eye(16, dtype=np.float32)
    idbf = np.eye(128).astype(ml_dtypes.bfloat16)
    o16 = np.ones((16, 1), np.float32)
    f = np.arange(CF)
    g = np.arange(8)
    mk = np.zeros((CF, 128), np.float32)
    mk.reshape(CF, 8, 16)[:, :, :] = (f[:, None] % 8 == g[None, :]).astype(
        np.float32)[:, :, None]
    rsel = (f[:, None] // 8 == np.arange(CS)[None, :]).astype(np.float32)
    return dict(blk128=blk, e1b=e1b, iotap1=iotap1, id4=id4, o416=o416,
                id16=id16, idbf=idbf, o16=o16, mk=mk, rsel=rsel)


def make_in_maps(inputs, cfg: Cfg = FULL):
    x = np.asarray(inputs["x"], np.float32).reshape(cfg.BT, cfg.D)
    Wg = np.ascontiguousarray(np.asarray(inputs["Wg"], np.float32))
    W1 = np.asarray(inputs["W1"], np.float32)
    W2 = np.asarray(inputs["W2"], np.float32)
    consts = host_consts(cfg)
    x_bf = x.astype(ml_dtypes.bfloat16)
    in_maps = []
    for i in range(NCORES):
        m = dict(consts)
        m["x_bf"] = x_bf
        m["xt_sh"] = np.ascontiguousarray(x[i * cfg.TSH:(i + 1) * cfg.TSH].T)
        m["wg"] = Wg
        m["w1"] = np.ascontiguousarray(W1[i].astype(ml_dtypes.bfloat16))
        m["w2"] = np.ascontiguousarray(W2[i].astype(ml_dtypes.bfloat16))
        in_maps.append(m)
    return in_maps


def assemble_out(results, cfg: Cfg = FULL):
    nf = np.stack([np.asarray(results[i]["nf_out"]) for i in range(NCORES)])
    if not (nf == cfg.C).all():
        print(f"WARNING: sparse_gather num_found != {cfg.C}: {nf.tolist()}",
              file=sys.stderr)
    TQ = 1024
    out = np.empty((cfg.B, cfg.T, cfg.D), np.float32)
    for i in range(NCORES):
        r = np.asarray(results[i]["out_sh"]).reshape(cfg.B, TQ, cfg.D)
        for b in range(cfg.B):
            out[b, i * TQ:(i + 1) * TQ] = r[b]
    return out


_NC_CACHE = {}


def get_nc():
    if "nc" not in _NC_CACHE:
        _NC_CACHE["nc"] = build_nc(_NC_CACHE.get("cfg", FULL))
    return _NC_CACHE["nc"]


def kernel(**inputs):
    nc = get_nc()
    in_maps = make_in_maps(inputs, _NC_CACHE.get("cfg", FULL))
    res = run_bass_kernel_spmd(nc, in_maps, core_ids=list(range(NCORES)),
                               **_NC_CACHE.get("run_kwargs", {}))
    _NC_CACHE["last_run"] = res
    return assemble_out(res.results, FULL)


# revision 76
# speedup vs baseline: 1.1646x; 1.1312x over previous
"""Expert-choice MoE router kernel for Trainium2 (8 NeuronCores), v3.

Problem (B=4, T=8192, D=512, E=8, H=2048, C=1024):
  scores = x @ Wg; w = softmax over T per (b,e); top-C tokens per (b,e);
  y = gelu(x[sel] @ W1) @ W2 * w[sel]; out = scatter_add(y)/max(sum w, 1e-8)

Sharding: expert-parallel, one expert per core.

v3 = v1's proven selection/compaction machinery + the v2 wins that were
individually validated on HW:
  - dense accumulator: bf16, split per batch (4 tensors), zeroed on the
    scalar-engine DMA queue (v1 lost 200 us blocking the sync queue).
  - ReduceScatter: bf16, per batch, issued inside the FFN loop so RS(b)
    overlaps FFN(b+1).
  - bisection: 28 rounds over [-8, 8] (max|score| ~4.9, top-C gap 3.7e-6
    >> 16/2^28), one PE trip per round via block-diag segment-sum.
  - compaction: v1 verbatim (two sparse_gathers per batch on (16, 512)
    slices + selection-matmul relayout) -- the v2 paired variant crashes
    the exec unit on HW.
"""

import sys
from dataclasses import dataclass

sys.path.insert(0, "/opt/trn_rl_repo")

import numpy as np
import ml_dtypes

import concourse.bass as bass  # noqa: F401
import concourse.mybir as mybir
import concourse.tile as tile
from concourse import bacc
from concourse.bass import IndirectOffsetOnAxis
from concourse.bass_utils import run_bass_kernel_spmd

F32 = mybir.dt.float32
BF16 = mybir.dt.bfloat16
I32 = mybir.dt.int32
U32 = mybir.dt.uint32
AF = mybir.ActivationFunctionType
ALU = mybir.AluOpType

NCORES = 8


@dataclass(frozen=True)
class Cfg:
    B: int = 4
    T: int = 8192
    D: int = 512
    E: int = 8
    H: int = 2048
    C: int = 1024
    nrounds: int = 28
    span: float = 8.0
    act: str = "Gelu"
    acc_bf16: bool = True
    rs_inline: bool = True
    stage: int = 4   # 1=+bisect, 2=+selection, 3=+FFN, 4=full

    @property
    def BT(self):
        return self.B * self.T

    @property
    def TSH(self):
        return self.BT // NCORES

    @property
    def ROW(self):
        return self.D + 8

    @property
    def DC(self):
        return self.D // 128

    @property
    def HC(self):
        return self.H // 128

    @property
    def TPP(self):
        return self.T * self.B // 128

    @property
    def RPB(self):
        return NCORES // self.B

    @property
    def QL(self):
        return self.T // self.RPB // 16

    @property
    def TB16(self):
        return self.T // 16

    @property
    def CF(self):
        return self.C // 16

    @property
    def CS(self):
        return self.C // 128


FULL = Cfg()


def build_nc(cfg: Cfg = FULL):
    B, T, D, E, H, C = cfg.B, cfg.T, cfg.D, cfg.E, cfg.H, cfg.C
    BT, TSH, ROW, DC, HC = cfg.BT, cfg.TSH, cfg.ROW, cfg.DC, cfg.HC
    TPP, RPB, QL, TB16 = cfg.TPP, cfg.RPB, cfg.QL, cfg.TB16
    CF, CS = cfg.CF, cfg.CS
    NT = 512
    TQ = 1024               # rows per core of a per-batch ReduceScatter
    ACC = BF16 if cfg.acc_bf16 else F32

    nc = bacc.Bacc("TRN2", target_bir_lowering=False, debug=False,
                   num_devices=NCORES)

    # ---- I/O ----
    x_bf = nc.dram_tensor("x_bf", [BT, D], BF16, kind="ExternalInput")
    xt_sh = nc.dram_tensor("xt_sh", [D, TSH], F32, kind="ExternalInput")
    wg_d = nc.dram_tensor("wg", [D, E], F32, kind="ExternalInput")
    w1_d = nc.dram_tensor("w1", [D, H], BF16, kind="ExternalInput")
    w2_d = nc.dram_tensor("w2", [H, D], BF16, kind="ExternalInput")
    blk_d = nc.dram_tensor("blk128", [128, 128], F32, kind="ExternalInput")
    e1b_d = nc.dram_tensor("e1b", [128, B], F32, kind="ExternalInput")
    iotap1_d = nc.dram_tensor("iotap1", [16, B * TB16], F32,
                              kind="ExternalInput")
    id4_d = nc.dram_tensor("id4", [B, B], I32, kind="ExternalInput")
    o416_d = nc.dram_tensor("o416", [B, 16], F32, kind="ExternalInput")
    id16_d = nc.dram_tensor("id16", [16, 16], F32, kind="ExternalInput")
    idbf_d = nc.dram_tensor("idbf", [128, 128], BF16, kind="ExternalInput")
    o16_d = nc.dram_tensor("o16", [16, 1], F32, kind="ExternalInput")
    mk_d = nc.dram_tensor("mk", [CF, 128], F32, kind="ExternalInput")
    rsel_d = nc.dram_tensor("rsel", [CF, CS], F32, kind="ExternalInput")

    out_sh = nc.dram_tensor("out_sh", [TSH, D], F32, kind="ExternalOutput")
    nf_out = nc.dram_tensor("nf_out", [B, 2], U32, kind="ExternalOutput")

    # ---- internal DRAM ----
    a2a_in = nc.dram_tensor("a2a_in", [E, TSH], F32)
    a2a_out = nc.dram_tensor("a2a_out", [E, TSH], F32)
    dense_b = [nc.dram_tensor(f"dense{b}", [T, ROW], ACC) for b in range(B)]
    rs_b = [nc.dram_tensor(f"rs{b}", [TQ, ROW], ACC) for b in range(B)]

    with tile.TileContext(nc) as tc:
        with (
            tc.tile_pool(name="const", bufs=1) as cp,
            tc.tile_pool(name="sc", bufs=2) as scp,
            tc.tile_pool(name="bis", bufs=1) as bp,
            tc.tile_pool(name="ffn", bufs=2) as fp,
            tc.tile_pool(name="pk", bufs=2 if cfg.acc_bf16 else 1) as pkp,
            tc.tile_pool(name="norm", bufs=2) as np_,
            tc.tile_pool(name="pmm", bufs=4, space="PSUM") as pmm,
            tc.tile_pool(name="pps", bufs=2, space="PSUM") as pps,
        ):
            # ---------- phase 0: zero dense accumulators (scalar queue) ---
            ZR = 8 if cfg.acc_bf16 else 4
            zt = cp.tile([128, ZR * ROW], ACC, tag="zt")
            nc.vector.memset(zt[:], 0.0)
            for b in range(B):
                dz = dense_b[b].ap().rearrange(
                    "(j p zr) r -> j p (zr r)", p=128, zr=ZR)
                for j in range(T // (128 * ZR)):
                    nc.scalar.dma_start(dz[j], zt[:])

            # ---------- load constants / weights ----------
            wg_sb = cp.tile([128, DC, E], F32, tag="wg_sb")
            nc.sync.dma_start(wg_sb[:], wg_d.ap().rearrange("(c p) e -> p c e", p=128))
            w1_sb = cp.tile([128, DC, H], BF16, tag="w1_sb")
            nc.sync.dma_start(w1_sb[:], w1_d.ap().rearrange("(c p) h -> p c h", p=128))
            w2_sb = cp.tile([128, HC, D], BF16, tag="w2_sb")
            nc.sync.dma_start(w2_sb[:], w2_d.ap().rearrange("(c p) d -> p c d", p=128))
            blks = cp.tile([128, 128], F32, tag="blks")
            nc.sync.dma_start(blks[:], blk_d.ap())
            e1bs = cp.tile([128, B], F32, tag="e1bs")
            nc.sync.dma_start(e1bs[:], e1b_d.ap())
            iotap1 = cp.tile([16, B * TB16], F32, tag="iotap1")
            nc.sync.dma_start(iotap1[:], iotap1_d.ap())
            id4s = cp.tile([B, B], I32, tag="id4s")
            nc.sync.dma_start(id4s[:], id4_d.ap())
            o416s = cp.tile([B, 16], F32, tag="o416s")
            nc.sync.dma_start(o416s[:], o416_d.ap())
            id16s = cp.tile([16, 16], F32, tag="id16s")
            nc.sync.dma_start(id16s[:], id16_d.ap())
            idbfs = cp.tile([128, 128], BF16, tag="idbfs")
            nc.sync.dma_start(idbfs[:], idbf_d.ap())
            o16s = cp.tile([16, 1], F32, tag="o16s")
            nc.sync.dma_start(o16s[:], o16_d.ap())
            mks = cp.tile([CF, 128], F32, tag="mks")
            nc.sync.dma_start(mks[:], mk_d.ap())
            rsels = cp.tile([CF, CS], F32, tag="rsels")
            nc.sync.dma_start(rsels[:], rsel_d.ap())

            # ---------- phase 1: partial scores for my token shard -------
            for nt in range(TSH // 512):
                xt_t = scp.tile([128, DC, 512], F32, tag="xt")
                nc.sync.dma_start(
                    xt_t[:],
                    xt_sh.ap().rearrange("(c p) t -> p c t", p=128)[
                        :, :, nt * 512:(nt + 1) * 512],
                )
                ps_sc = pps.tile([E, 512], F32, tag="sp")
                for dc in range(DC):
                    nc.tensor.matmul(ps_sc[:], lhsT=wg_sb[:, dc, :],
                                     rhs=xt_t[:, dc, :],
                                     start=(dc == 0), stop=(dc == DC - 1))
                sc_sb = scp.tile([E, 512], F32, tag="scsb")
                nc.vector.tensor_copy(sc_sb[:], ps_sc[:])
                nc.sync.dma_start(a2a_in[:, nt * 512:(nt + 1) * 512], sc_sb[:])

            # ---------- phase 2: AllToAll ----------
            nc.gpsimd.collective_compute(
                "AllToAll", ALU.bypass, replica_groups=[list(range(NCORES))],
                ins=[a2a_in.ap()], outs=[a2a_out.ap()],
            )

            PPR = 128 // E
            w128 = cp.tile([128, TPP], F32, tag="w128")
            for r in range(E):
                nc.sync.dma_start(
                    w128[r * PPR:(r + 1) * PPR, :],
                    a2a_out.ap()[r].rearrange("(l f) -> l f", l=PPR))
            w16 = cp.tile([16, B * TB16], F32, tag="w16")
            for r in range(E):
                b, q = divmod(r, RPB)
                nc.sync.dma_start(
                    w16[:, b * TB16 + q * QL: b * TB16 + (q + 1) * QL],
                    a2a_out.ap()[r].rearrange("(s j) -> s j", s=16))

            # ---------- phase 3: softmax pieces ----------
            exp16 = cp.tile([16, B * TB16], F32, tag="exp16")
            parts16 = bp.tile([16, B], F32, tag="parts16")
            for b in range(B):
                sl = slice(b * TB16, (b + 1) * TB16)
                nc.scalar.activation(exp16[:, sl], w16[:, sl], AF.Exp,
                                     accum_out=parts16[:, b:b + 1])
            ps4 = pps.tile([B, 1], F32, tag="sp")
            nc.tensor.matmul(ps4[:], lhsT=parts16[:], rhs=o16s[:],
                             start=True, stop=True)
            recip4 = bp.tile([B, 1], F32, tag="recip4")
            nc.vector.reciprocal(recip4[:], ps4[:])
            diagr = bp.tile([B, B], F32, tag="diagr")
            nc.vector.memset(diagr[:], 0.0)
            nc.vector.copy_predicated(diagr[:], id4s[:],
                                      recip4[:, 0:1].to_broadcast([B, B]))
            psr16 = pps.tile([16, B], F32, tag="sp")
            nc.tensor.matmul(psr16[:], lhsT=o416s[:], rhs=diagr[:],
                             start=True, stop=True)
            recip16 = cp.tile([16, B], F32, tag="recip16")
            nc.vector.tensor_copy(recip16[:], psr16[:])

            # ---------- phase 4: threshold bisection ----------
            lo128 = bp.tile([128, 1], F32, tag="lo128")
            hi128 = bp.tile([128, 1], F32, tag="hi128")
            nc.vector.memset(lo128[:], -cfg.span)
            nc.vector.memset(hi128[:], cfg.span)
            mid128 = bp.tile([128, 1], F32, tag="mid128")
            sel128 = bp.tile([128, 1], I32, tag="sel128")
            seli128 = bp.tile([128, 1], I32, tag="seli128")
            cnt128 = bp.tile([128, 1], F32, tag="cnt128")
            msk = bp.tile([128, TPP], F32, tag="msk")
            for _ in range(cfg.nrounds if cfg.stage >= 1 else 0):
                nc.vector.tensor_add(mid128[:], lo128[:], hi128[:])
                nc.vector.tensor_scalar_mul(mid128[:], mid128[:], 0.5)
                nc.vector.tensor_scalar(msk[:], w128[:], mid128[:, 0:1], None,
                                        op0=ALU.is_ge, op1=ALU.add,
                                        accum_out=cnt128[:, 0:1])
                ptot = pps.tile([128, 1], F32, tag="sp")
                nc.tensor.matmul(ptot[:], lhsT=blks[:], rhs=cnt128[:],
                                 start=True, stop=True)
                nc.vector.tensor_scalar(sel128[:], ptot[:], float(C) - 0.5,
                                        None, op0=ALU.is_ge)
                nc.vector.tensor_scalar(seli128[:], ptot[:], float(C) - 0.5,
                                        None, op0=ALU.is_lt)
                nc.vector.copy_predicated(lo128[:], sel128[:], mid128[:])
                nc.vector.copy_predicated(hi128[:], seli128[:], mid128[:])

            # tau4 / tau16
            ptau = pps.tile([B, 1], F32, tag="sp")
            nc.tensor.matmul(ptau[:], lhsT=e1bs[:], rhs=lo128[:],
                             start=True, stop=True)
            tau4 = bp.tile([B, 1], F32, tag="tau4")
            nc.vector.tensor_copy(tau4[:], ptau[:])
            diagt = bp.tile([B, B], F32, tag="diagt")
            nc.vector.memset(diagt[:], 0.0)
            nc.vector.copy_predicated(diagt[:], id4s[:],
                                      tau4[:, 0:1].to_broadcast([B, B]))
            pst16 = pps.tile([16, B], F32, tag="sp")
            nc.tensor.matmul(pst16[:], lhsT=o416s[:], rhs=diagt[:],
                             start=True, stop=True)
            tau16 = cp.tile([16, B], F32, tag="tau16")
            nc.vector.tensor_copy(tau16[:], pst16[:])

            # ---------- phase 5: compaction + 16->128 relayout (v1) ------
            idxg32s, idxl32s, val128s, selTs = [], [], [], []
            for b in range(B if cfg.stage >= 2 else 0):
                sl = slice(b * TB16, (b + 1) * TB16)
                mask16 = bp.tile([16, TB16], F32, tag="mask16")
                nc.vector.tensor_scalar(mask16[:], w16[:, sl], tau16[:, b:b + 1],
                                        None, op0=ALU.is_ge)
                candi = bp.tile([16, TB16], F32, tag="candi")
                nc.vector.tensor_tensor(candi[:], mask16[:], iotap1[:, sl],
                                        op=ALU.mult)
                nc.vector.tensor_scalar_add(candi[:], candi[:], -1.0)
                candv = bp.tile([16, TB16], F32, tag="candv")
                nc.vector.tensor_tensor(candv[:], mask16[:], exp16[:, sl],
                                        op=ALU.mult)
                nc.vector.tensor_scalar_add(mask16[:], mask16[:], -1.0)
                nc.vector.tensor_tensor(candv[:], candv[:], mask16[:],
                                        op=ALU.add)

                ci = bp.tile([16, CF + 16], F32, tag=f"ci{b}")
                nfi = bp.tile([1, 1], U32, tag=f"nfi{b}")
                nc.gpsimd.sparse_gather(ci[:], candi[:], num_found=nfi[:])
                cv = bp.tile([16, CF + 16], F32, tag=f"cv{b}")
                nfv = bp.tile([1, 1], U32, tag=f"nfv{b}")
                nc.gpsimd.sparse_gather(cv[:], candv[:], num_found=nfv[:])
                nc.sync.dma_start(nf_out.ap()[b:b + 1, 0:1], nfi[:, :])
                nc.sync.dma_start(nf_out.ap()[b:b + 1, 1:2], nfv[:, :])

                nc.vector.tensor_scalar(cv[:, :CF], cv[:, :CF],
                                        recip16[:, b:b + 1], None, op0=ALU.mult)
                nc.vector.tensor_scalar_add(ci[:, :CF], ci[:, :CF],
                                            float(b * T))

                pti = pps.tile([CF, 16], F32, tag="sp")
                nc.tensor.transpose(pti[:], ci[:, :CF], id16s[:])
                cit = bp.tile([CF, 16], F32, tag="cit")
                nc.vector.tensor_copy(cit[:], pti[:])
                ptv = pps.tile([CF, 16], F32, tag="sp")
                nc.tensor.transpose(ptv[:], cv[:, :CF], id16s[:])
                cvt = bp.tile([CF, 16], F32, tag="cvt")
                nc.vector.tensor_copy(cvt[:], ptv[:])

                cmi = bp.tile([CF, 128], F32, tag="cmi")
                nc.vector.tensor_tensor(
                    cmi[:].rearrange("f (g s) -> f g s", g=8),
                    cit[:, None, :].to_broadcast([CF, 8, 16]),
                    mks[:].rearrange("f (g s) -> f g s", g=8),
                    op=ALU.mult)
                cmv = bp.tile([CF, 128], F32, tag="cmv")
                nc.vector.tensor_tensor(
                    cmv[:].rearrange("f (g s) -> f g s", g=8),
                    cvt[:, None, :].to_broadcast([CF, 8, 16]),
                    mks[:].rearrange("f (g s) -> f g s", g=8),
                    op=ALU.mult)

                pri = pps.tile([128, CS], F32, tag="sp")
                nc.tensor.matmul(pri[:], lhsT=cmi[:], rhs=rsels[:],
                                 start=True, stop=True)
                idxg32 = cp.tile([128, CS], I32, name=f"idxg_{b}",
                                 tag=f"idxg_{b}")
                nc.vector.tensor_scalar(idxg32[:], pri[:], 0.0, None,
                                        op0=ALU.max)
                idxl32 = cp.tile([128, CS], I32, name=f"idxl_{b}",
                                 tag=f"idxl_{b}")
                nc.vector.tensor_scalar(idxl32[:], pri[:], float(-b * T),
                                        None, op0=ALU.add)
                prv = pps.tile([128, CS], F32, tag="sp")
                nc.tensor.matmul(prv[:], lhsT=cmv[:], rhs=rsels[:],
                                 start=True, stop=True)
                val128 = cp.tile([128, CS], F32, name=f"val_{b}",
                                 tag=f"val_{b}")
                nc.vector.tensor_copy(val128[:], prv[:])
                idxg32s.append(idxg32)
                idxl32s.append(idxl32)
                val128s.append(val128)

                # gather + transpose now, BEFORE any ReduceScatter is in
                # flight: PE transposes are serialized against collectives
                # by the tile framework, which stalled the FFN ~50us/batch.
                if cfg.stage >= 3:
                    selTM = fp.tile([128, CS, D], BF16, tag="selTM")
                    for cs in range(CS):
                        nc.gpsimd.indirect_dma_start(
                            out=selTM[:, cs, :],
                            out_offset=None,
                            in_=x_bf.ap(),
                            in_offset=IndirectOffsetOnAxis(
                                ap=idxg32[:, cs:cs + 1], axis=0))
                    selT = cp.tile([128, DC, C], BF16, name=f"selT_{b}",
                                   tag=f"selT_{b}")
                    for cs in range(CS):
                        for dc in range(DC):
                            ptp = pps.tile([128, 128], BF16, tag="tp")
                            nc.tensor.transpose(
                                ptp[:], selTM[:, cs, dc * 128:(dc + 1) * 128],
                                idbfs[:])
                            nc.vector.tensor_copy(
                                selT[:, dc, cs * 128:(cs + 1) * 128], ptp[:])
                    selTs.append(selT)

            # ---------- phase 6: per-batch FFN + scatter + inline RS -----
            for b in range(B if cfg.stage >= 3 else 0):
                selT = selTs[b]
                pk = pkp.tile([128, CS, ROW], ACC, tag="pk")
                nc.gpsimd.memset(pk[:], 0.0)
                for ct in range(C // NT):
                    csl = slice(ct * NT, (ct + 1) * NT)
                    hT = fp.tile([128, HC, NT], BF16, tag="hT")
                    for ht in range(HC):
                        psh = pmm.tile([128, NT], F32, tag="mm")
                        for dc in range(DC):
                            nc.tensor.matmul(
                                psh[:],
                                lhsT=w1_sb[:, dc, ht * 128:(ht + 1) * 128],
                                rhs=selT[:, dc, csl],
                                start=(dc == 0), stop=(dc == DC - 1))
                        nc.scalar.activation(hT[:, ht, :], psh[:],
                                             getattr(AF, cfg.act))
                    for cl in range(NT // 128):
                        cs = ct * (NT // 128) + cl
                        pso = pmm.tile([128, D], F32, tag="mm")
                        for hc in range(HC):
                            nc.tensor.matmul(
                                pso[:],
                                lhsT=hT[:, hc, cl * 128:(cl + 1) * 128],
                                rhs=w2_sb[:, hc, :],
                                start=(hc == 0), stop=(hc == HC - 1))
                        # drain psum on the SCALAR engine: DVE drains here
                        # chain the PE to the stalled normalize ops via the
                        # in-order DVE queue (PSUM WAR edges)
                        nc.scalar.activation(pk[:, cs, :D], pso[:], AF.Copy,
                                             scale=val128s[b][:, cs:cs + 1])
                        nc.scalar.activation(pk[:, cs, D:D + 1],
                                             val128s[b][:, cs:cs + 1],
                                             AF.Copy)
                for cs in range(CS):
                    nc.gpsimd.indirect_dma_start(
                        out=dense_b[b].ap(),
                        out_offset=IndirectOffsetOnAxis(
                            ap=idxl32s[b][:, cs:cs + 1], axis=0),
                        in_=pk[:, cs, :],
                        in_offset=None,
                        bounds_check=T - 1,
                        oob_is_err=False)
                if cfg.rs_inline and cfg.stage >= 4:
                    nc.gpsimd.collective_compute(
                        "ReduceScatter", ALU.add,
                        replica_groups=[list(range(NCORES))],
                        ins=[dense_b[b].ap()], outs=[rs_b[b].ap()],
                    )
            if not cfg.rs_inline and cfg.stage >= 4:
                for b in range(B):
                    nc.gpsimd.collective_compute(
                        "ReduceScatter", ALU.add,
                        replica_groups=[list(range(NCORES))],
                        ins=[dense_b[b].ap()], outs=[rs_b[b].ap()],
                    )

            # ---------- phase 7: normalize my shard of each batch --------
            # Scheduled at the very end (priority pushed +1e6): its DVE ops
            # depend on RS outputs, and if interleaved into the FFN-era DVE
            # stream they stall the in-order DVE queue, which stalls the PE
            # via PSUM WAR edges (~47us per batch).
            with tc.high_priority(offset=-1000000):
                for b in range(B if cfg.stage >= 4 else 0):
                    for j in range(TQ // 128):
                        rsl = slice(j * 128, (j + 1) * 128)
                        ld = np_.tile([128, D + 1], ACC, tag="ld")
                        nc.sync.dma_start(ld[:], rs_b[b].ap()[rsl, :D + 1])
                        dn = np_.tile([128, 1], F32, tag="dn")
                        nc.vector.tensor_scalar(dn[:], ld[:, D:D + 1], 1e-8,
                                                None, op0=ALU.max)
                        rc = np_.tile([128, 1], F32, tag="rc")
                        nc.vector.reciprocal(rc[:], dn[:])
                        ot = np_.tile([128, D], F32, tag="ot")
                        nc.vector.tensor_scalar(ot[:], ld[:, :D], rc[:, 0:1],
                                                None, op0=ALU.mult)
                        nc.sync.dma_start(
                            out_sh.ap()[b * TQ + j * 128:
                                        b * TQ + (j + 1) * 128, :],
                            ot[:])

    nc.compile()
    return nc


# ---------------------------------------------------------------------------
# host side
# ---------------------------------------------------------------------------

def host_consts(cfg: Cfg = FULL):
    B, T = cfg.B, cfg.T
    TB16, RPB, QL, CF, CS = cfg.TB16, cfg.RPB, cfg.QL, cfg.CF, cfg.CS
    p = np.arange(128)
    blk = (p[:, None] // 32 == p[None, :] // 32).astype(np.float32)
    e1b = (p[:, None] // 32 == np.arange(B)[None, :]).astype(np.float32) / 32.0
    iotap1 = np.zeros((16, B * TB16), np.float32)
    for s in range(16):
        for q in range(RPB):
            j = np.arange(QL)
            t = q * (T // RPB) + s * QL + j
            for b in range(B):
                iotap1[s, b * TB16 + q * QL + j] = t + 1
    id4 = np.eye(B, dtype=np.int32)
    o416 = np.ones((B, 16), np.float32)
    id16 = np.eye(16, dtype=np.float32)
    idbf = np.eye(128).astype(ml_dtypes.bfloat16)
    o16 = np.ones((16, 1), np.float32)
    f = np.arange(CF)
    g = np.arange(8)
    mk = np.zeros((CF, 128), np.float32)
    mk.reshape(CF, 8, 16)[:, :, :] = (f[:, None] % 8 == g[None, :]).astype(
        np.float32)[:, :, None]
    rsel = (f[:, None] // 8 == np.arange(CS)[None, :]).astype(np.float32)
    return dict(blk128=blk, e1b=e1b, iotap1=iotap1, id4=id4, o416=o416,
                id16=id16, idbf=idbf, o16=o16, mk=mk, rsel=rsel)


def make_in_maps(inputs, cfg: Cfg = FULL):
    x = np.asarray(inputs["x"], np.float32).reshape(cfg.BT, cfg.D)
    Wg = np.ascontiguousarray(np.asarray(inputs["Wg"], np.float32))
    W1 = np.asarray(inputs["W1"], np.float32)
    W2 = np.asarray(inputs["W2"], np.float32)
    consts = host_consts(cfg)
    x_bf = x.astype(ml_dtypes.bfloat16)
    in_maps = []
    for i in range(NCORES):
        m = dict(consts)
        m["x_bf"] = x_bf
        m["xt_sh"] = np.ascontiguousarray(x[i * cfg.TSH:(i + 1) * cfg.TSH].T)
        m["wg"] = Wg
        m["w1"] = np.ascontiguousarray(W1[i].astype(ml_dtypes.bfloat16))
        m["w2"] = np.ascontiguousarray(W2[i].astype(ml_dtypes.bfloat16))
        in_maps.append(m)
    return in_maps


def assemble_out(results, cfg: Cfg = FULL):
    nf = np.stack([np.asarray(results[i]["nf_out"]) for i in range(NCORES)])
    if not (nf == cfg.C).all():
        print(f"WARNING: sparse_gather num_found != {cfg.C}: {nf.tolist()}",
              file=sys.stderr)
    TQ = 1024
    out = np.empty((cfg.B, cfg.T, cfg.D), np.float32)
    for i in range(NCORES):
        r = np.asarray(results[i]["out_sh"]).reshape(cfg.B, TQ, cfg.D)
        for b in range(cfg.B):
            out[b, i * TQ:(i + 1) * TQ] = r[b]
    return out


_NC_CACHE = {}


def get_nc():
    if "nc" not in _NC_CACHE:
        _NC_CACHE["nc"] = build_nc(_NC_CACHE.get("cfg", FULL))
    return _NC_CACHE["nc"]


def kernel(**inputs):
    nc = get_nc()
    in_maps = make_in_maps(inputs, _NC_CACHE.get("cfg", FULL))
    res = run_bass_kernel_spmd(nc, in_maps, core_ids=list(range(NCORES)),
                               **_NC_CACHE.get("run_kwargs", {}))
    _NC_CACHE["last_run"] = res
    return assemble_out(res.results, FULL)
